# revision 2
# baseline (speedup 1.0000x reference)
"""Trainium2 Bass kernel for causal masked-ReLU attention (no softmax).

Reference computation (B=8, T=1024, C=768, n_head=12, hd=64):
    qkv = x @ W_attn.T + b_attn
    q, k, v = split(qkv); per-head: att = relu(mask_causal(q k^T / sqrt(hd)))
    y = att @ v, heads re-merged -> (B, T, C)

Sharding: one batch element per NeuronCore (8 cores). Each core computes the
QKV projection and all 12 heads' attention for its batch element.

Layout strategy (per core):
  - Host passes x[b].T (C, T) and W.T (C, 3C) so the contraction dim C lands
    on SBUF partitions with unit-stride DMA (no on-chip transposes).
  - W rows are pre-permuted on host into [q-pair0, k-pair0, q-pair1, ...] so
    q.T / k.T of head h live at the same partition offset (h%2)*64 of their
    M-tiles; matmul operands then share a base partition.
  - q weights/bias are pre-scaled by 1/sqrt(hd) on host.
  - QKV projection runs in fp8 (e4m3) DoubleRow perf mode: 256-deep
    contraction per pass at 0.5 cycles/row = 4x fp16 PE throughput. Operands
    are split into hi+lo fp8 digits (x = xh + xl, W = wh + wl, both
    pre-scaled into e4m3's normal range) and three digit products
    xh*wh + xl*wh + xh*wl accumulate in the same fp32 PSUM group; the
    dropped xl*wl term is ~1e-4 relative. The 2^13 operand scaling is
    removed at eviction (activation scale / tensor_scalar multiply), where
    the bias is also added. Net error ~1.2e-3, PE cost 0.75x of fp16.
  - att is computed transposed (att.T = k @ q.T, layout [T_k, T_q]) so the AV
    matmul (y.T = v.T @ att.T) streams att.T directly with v as stationary.
    QK/AV run in fp16 (2-digit fp8 would double the eviction traffic; 1-digit
    fp8 error ~2.7% exceeds the tolerance).
  - Causal structure at 256-col granularity: fully-masked windows are never
    computed; att.T below-diagonal regions are zeroed once and never written.
  - Input DMAs are chained (~6 in flight) ordered so the v-projection's
    hi/lo k-pair tiles land first, then the q/k weight tiles.
  - QK evictions and AV accumulation groups both run ascending, so att
    tiles are read and released in the same order the next head rewrites
    them, and the QK eviction stream ends on the cheap diagonal tiles.
  - Eviction work (masked-ReLU, bias adds, y copies) is balanced across
    the ACT and DVE engines, which sit just under the PE's per-head time.
  - A few dummy matmuls during the initial DMA wait keep the PE's HAM
    activity window warm so real matmuls start at the full 2.4 GHz clock.
  - Output is written as y.T (C, T) in fp16; host transposes and upcasts.
"""

import numpy as np

import sys
for _p in ("/opt/trn_rl_repo", "/root/.axon_site", "/root/.axon_site/_ro/trn_rl_repo",
           "/root/.axon_site/_ro/pypackages"):
    if _p not in sys.path:
        sys.path.append(_p)

import ml_dtypes

import concourse.bacc as bacc
import concourse.mybir as mybir
from concourse.alu_op_type import AluOpType
from concourse.tile import TileContext
from concourse.tile_rust import add_dep_helper
from concourse.bass_utils import run_bass_kernel_spmd

B, T, C = 8, 1024, 768
NH, HD = 12, 64
C3 = 3 * C            # 2304
KT = C // 128         # 6  contraction tiles of the projection
NP = KT // 2          # 3  contraction pairs (DoubleRow)
TT = T // 128         # 8  tiles of the sequence dim
NPAIR = NH // 2       # 6  head pairs
NW = T // 256         # 4  256-wide attention windows
F32 = mybir.dt.float32
F16 = mybir.dt.float16
F8 = mybir.dt.float8e4
AF = mybir.ActivationFunctionType
DR = mybir.MatmulPerfMode.DoubleRow

SX = 16.0             # x pre-scale (keeps x-lo digits in e4m3 normal range)
SW = 512.0            # W pre-scale
DESCALE = 1.0 / (SX * SW)

WARM_MMS = 8

_CACHE = {}


def _build():
    nc = bacc.Bacc("TRN2", target_bir_lowering=False, debug=False, num_devices=8)

    xh = nc.dram_tensor("xh", [C, T], F8, kind="ExternalInput").ap()
    xl = nc.dram_tensor("xl", [C, T], F8, kind="ExternalInput").ap()
    wh = nc.dram_tensor("wh", [C, C3], F8, kind="ExternalInput").ap()
    wl = nc.dram_tensor("wl", [C, C3], F8, kind="ExternalInput").ap()
    bqk = nc.dram_tensor("bqk", [128, 2 * NPAIR], F32, kind="ExternalInput").ap()
    bvb = nc.dram_tensor("bvb", [128, C], F32, kind="ExternalInput").ap()
    # masks = [tri(128) | ones(896)]: the kept region of att.T tile tk always
    # starts with the triangular diagonal block, so masks[:, :width] is the
    # relu-mask for any kept window
    masks = nc.dram_tensor("masks", [128, T], F32, kind="ExternalInput").ap()
    zeros = nc.dram_tensor("zeros", [128, T - 128], F16, kind="ExternalInput").ap()
    yT = nc.dram_tensor("yT", [C, T], F16, kind="ExternalOutput").ap()

    dma_chain = []
    CHAIN_DEPTH = 6

    def chained_dma(out, in_):
        # keep ~CHAIN_DEPTH input DMAs in flight: enough to pipeline the DMA
        # queues, few enough that k-tiles still arrive roughly in issue order
        inst = nc.sync.dma_start(out=out, in_=in_)
        if len(dma_chain) >= CHAIN_DEPTH:
            add_dep_helper(inst.ins, dma_chain[-CHAIN_DEPTH].ins, True,
                           "dma staging chain")
        dma_chain.append(inst)
        return inst

    with TileContext(nc) as tc:
        with (
            tc.tile_pool(name="persist", bufs=1) as pp,
            tc.tile_pool(name="psum_y", bufs=2, space="PSUM") as ps_y,
        ):
            masks_sb = pp.tile([128, T], F32, name="masks_sb")
            bqk_sb = pp.tile([128, 2 * NPAIR], F32, name="bqk_sb")
            bvb_sb = pp.tile([128, C], F32, name="bvb_sb")
            qkT = [pp.tile([128, T], F16, name=f"qkT{m}") for m in range(2 * NPAIR)]
            v_sb = [pp.tile([128, C], F16, name=f"v{t}") for t in range(TT)]
            attsets = [[pp.tile([128, T], F16, name=f"att{s}_{t}")
                        for t in range(TT)] for s in range(2)]

            # ---------- Phase 1: QKV projection (fp8 DoubleRow, 3 digit
            # products xh*wh + xl*wh + xh*wl into one PSUM group) ----------
            with (
                tc.tile_pool(name="io", bufs=1) as iop,
                tc.tile_pool(name="psum_proj", bufs=6, space="PSUM") as ps_proj,
            ):
                # k-PAIR tiles: dim1 indexes the two 128-deep contraction
                # sub-tiles a DoubleRow matmul consumes per pass
                xh_sb = [iop.tile([128, 2, T], F8, name=f"xh{p}") for p in range(NP)]
                xl_sb = [iop.tile([128, 2, T], F8, name=f"xl{p}") for p in range(NP)]
                wh_sb = [iop.tile([128, 2, C3], F8, name=f"wh{p}") for p in range(NP)]
                wl_sb = [iop.tile([128, 2, C3], F8, name=f"wl{p}") for p in range(NP)]

                # input DMAs: per k-pair, x hi/lo + the v-slice of W hi/lo
                # first (v groups run first), then the q/k W slices, so the
                # PE can start the v windows after ~2 pairs land
                smalls = [(bvb_sb, bvb), (bqk_sb, bqk)]
                for p in range(NP):
                    for j in range(2):
                        k = 2 * p + j
                        chained_dma(xh_sb[p][:, j, :], xh[128 * k:128 * (k + 1), :])
                        chained_dma(wh_sb[p][:, j, 2 * C:],
                                    wh[128 * k:128 * (k + 1), 2 * C:])
                        chained_dma(xl_sb[p][:, j, :], xl[128 * k:128 * (k + 1), :])
                        chained_dma(wl_sb[p][:, j, 2 * C:],
                                    wl[128 * k:128 * (k + 1), 2 * C:])
                    if smalls:
                        dst, src_ = smalls.pop(0)
                        chained_dma(dst[:], src_[:])
                for p in range(NP):
                    for j in range(2):
                        k = 2 * p + j
                        chained_dma(wh_sb[p][:, j, :2 * C],
                                    wh[128 * k:128 * (k + 1), :2 * C])
                        chained_dma(wl_sb[p][:, j, :2 * C],
                                    wl[128 * k:128 * (k + 1), :2 * C])
                chained_dma(masks_sb[:], masks[:])
                # below-diagonal regions of att stay zero for the whole
                # kernel; evictions only ever write cols >= 128*t
                for s in range(2):
                    for t in range(1, TT):
                        chained_dma(attsets[s][t][:, :128 * t],
                                    zeros[:, :128 * t])

                # PE warmup: dummy matmuls on a never-written scratch tile
                # during the initial DMA wait; keeps the HAM activity window
                # busy so the real matmuls start at full clock. Results (and
                # operand garbage) are discarded.
                scratch = iop.tile([128, 512], F16, name="warm_src")
                nc.vector.memset(scratch[:], 0.0)
                warm = ps_proj.tile([128, 512], F32, name="ps_warm", tag="ps_proj")
                for _ in range(WARM_MMS):
                    nc.tensor.matmul(warm[:], scratch[:, :128], scratch[:],
                                     start=True, stop=True)

                # each group = one [128, 512] PSUM tile (one full bank / zero
                # region) holding one or two 256-wide DoubleRow chunks.
                # ("v", t, n0, width) / ("qk", m, q0, width)
                groups = []
                for t in range(TT):
                    groups.append(("v", t, 0, 512))
                    groups.append(("v", t, 512, 256))
                for m in range(2 * NPAIR):
                    for q0 in (0, 512):
                        groups.append(("qk", m, q0, 512))

                # windows of 5 psum tiles; digit-product-major, k-pair-minor
                # within the window so the PE consumes pairs as they land.
                # Term order hh, lh, hl matches the DMA order (xl before wl).
                for w0 in range(0, len(groups), 5):
                    window = groups[w0:w0 + 5]
                    tiles = [ps_proj.tile([128, 512], F32, name="ps_proj",
                                          tag="ps_proj") for _ in window]
                    nmm = {id(ps): 0 for ps in tiles}
                    total = {id(ps): 9 * (g[3] // 256)
                             for g, ps in zip(window, tiles)}
                    for term in range(3):
                        xa = (xh_sb, xl_sb, xh_sb)[term]
                        wa = (wh_sb, wh_sb, wl_sb)[term]
                        for p in range(NP):
                            for g, ps in zip(window, tiles):
                                kind, i, o0, wd = g
                                for c0 in range(0, wd, 256):
                                    n = nmm[id(ps)]
                                    nmm[id(ps)] = n + 1
                                    st = n == 0
                                    sp = n == total[id(ps)] - 1
                                    if kind == "v":
                                        nc.tensor.matmul(
                                            ps[:, c0:c0 + 256],
                                            xa[p][:, :, 128 * i:128 * (i + 1)],
                                            wa[p][:, :, 2 * C + o0 + c0:
                                                  2 * C + o0 + c0 + 256],
                                            start=st, stop=sp, perf_mode=DR,
                                        )
                                    else:
                                        nc.tensor.matmul(
                                            ps[:, c0:c0 + 256],
                                            wa[p][:, :, 128 * i:128 * (i + 1)],
                                            xa[p][:, :, o0 + c0:o0 + c0 + 256],
                                            start=st, stop=sp, perf_mode=DR,
                                        )
                    for g, ps in zip(window, tiles):
                        kind, i, o0, wd = g
                        if kind == "v":
                            nc.vector.scalar_tensor_tensor(
                                v_sb[i][:, o0:o0 + wd], ps[:, :wd], DESCALE,
                                bvb_sb[:, o0:o0 + wd],
                                AluOpType.mult, AluOpType.add,
                            )
                        elif i % 2 == 0:
                            nc.scalar.activation(
                                qkT[i][:, o0:o0 + wd], ps[:, :wd],
                                AF.Identity, bias=bqk_sb[:, i:i + 1],
                                scale=DESCALE,
                            )
                        else:
                            nc.vector.tensor_scalar(
                                qkT[i][:, o0:o0 + wd], ps[:, :wd],
                                DESCALE, bqk_sb[:, i:i + 1],
                                AluOpType.mult, AluOpType.add,
                            )

            # ---------- Phase 2: attention, head by head ----------
            with (
                tc.tile_pool(name="psum_att", bufs=6, space="PSUM") as ps_att,
                tc.tile_pool(name="yout", bufs=2) as yop,
            ):
                for a in range(NPAIR):
                    y_pair = yop.tile([128, T], F16, name="y_pair", tag="y_pair")
                    for phase in range(2):   # 0 = QK both heads, 1 = AV both heads
                      for r in range(2):
                        h = 2 * a + r
                        att = attsets[r]
                        qh = qkT[2 * a][64 * r:64 * (r + 1), :]
                        kh = qkT[2 * a + 1][64 * r:64 * (r + 1), :]
                        if phase == 0:
                          # QK^T -> att.T, tk ascending, single-bank psum tiles
                          # for fine-grained slot recycling. Ascending order
                          # ends on the cheap evictions (tk6/7 have no relu),
                          # so AV never waits behind an eviction backlog.
                          for tk in range(TT):
                              k0 = 128 * tk
                              q0d = 256 * (tk // 2)       # start of diag window
                              # piece 1: [k0, 512) if the diag sits left of 512
                              if q0d < 512:
                                  tag = "ps_y" if (h == 0 and tk <= 1) else "ps_att"
                                  pool = ps_y if tag == "ps_y" else ps_att
                                  ps = pool.tile([128, 512], F32,
                                                 name="ps_qk", tag=tag)
                                  pw = 512 - q0d
                                  nc.tensor.matmul(
                                      ps[:, k0 - q0d:pw], kh[:, k0:k0 + 128],
                                      qh[:, k0:512], start=True, stop=True,
                                  )
                                  # whole piece in one DVE op: relu * [tri|1..]
                                  nc.vector.scalar_tensor_tensor(
                                      att[tk][:, k0:512],
                                      ps[:, k0 - q0d:pw],
                                      0.0, masks_sb[:, :512 - k0],
                                      AluOpType.max, AluOpType.mult,
                                  )
                                  # piece 2: the full [512, 1024) half
                                  ps = pool.tile([128, 512], F32,
                                                 name="ps_qk", tag=tag)
                                  nc.tensor.matmul(
                                      ps[:], kh[:, k0:k0 + 128], qh[:, 512:T],
                                      start=True, stop=True,
                                  )
                                  nc.scalar.activation(att[tk][:, 512:T], ps[:],
                                                       AF.Relu)
                              else:
                                  # single piece [k0, 1024)
                                  ps = ps_att.tile([128, 512], F32,
                                                   name="ps_qk", tag="ps_att")
                                  pw = T - q0d
                                  nc.tensor.matmul(
                                      ps[:, k0 - q0d:pw], kh[:, k0:k0 + 128],
                                      qh[:, k0:T], start=True, stop=True,
                                  )
                                  nc.vector.scalar_tensor_tensor(
                                      att[tk][:, k0:T],
                                      ps[:, k0 - q0d:pw],
                                      0.0, masks_sb[:, :T - k0],
                                      AluOpType.max, AluOpType.mult,
                                  )

                        if phase == 1:
                          # AV: y.T = v.T @ att.T, groups j ascending, paired
                          # into one [64, 512] psum tile per half; att tiles are
                          # read and released in the order the next head's QK
                          # rewrites them
                          jhis = (3, 1) if h == NH - 1 else (1, 3)
                          for jhi in jhis:
                              ps2 = ps_y.tile([64, 512], F32, name="ps_av",
                                              tag="ps_y")
                              for j in (jhi - 1, jhi):
                                  q0 = 256 * j
                                  c0 = 256 * (j - (jhi - 1))
                                  ntk = min(TT, 2 * j + 2)
                                  for tk in range(ntk):
                                      lo = 128 if tk == 2 * j + 1 else 0
                                      nc.tensor.matmul(
                                          ps2[:, c0 + lo:c0 + 256],
                                          v_sb[tk][:, 64 * h:64 * (h + 1)],
                                          att[tk][:, q0 + lo:q0 + 256],
                                          start=(tk == 0), stop=(tk == ntk - 1),
                                      )
                              nc.scalar.copy(
                                  y_pair[64 * r:64 * (r + 1),
                                         256 * (jhi - 1):256 * (jhi + 1)],
                                  ps2[:],
                              )
                              nc.sync.dma_start(
                                  out=yT[128 * a + 64 * r:128 * a + 64 * (r + 1),
                                         256 * (jhi - 1):256 * (jhi + 1)],
                                  in_=y_pair[64 * r:64 * (r + 1),
                                             256 * (jhi - 1):256 * (jhi + 1)])

    nc.compile()
    return nc

def _prep_host(x, W_attn, b_attn):
    s = 1.0 / np.sqrt(np.float32(HD))
    W = np.asarray(W_attn, dtype=np.float32).copy()
    b = np.asarray(b_attn, dtype=np.float32).copy()
    W[:C] *= s
    b[:C] *= s
    # interleave q/k head pairs: [q-pair0, k-pair0, q-pair1, k-pair1, ...], v natural
    rows = []
    for a in range(NPAIR):
        rows.extend(range(128 * a, 128 * (a + 1)))          # q heads 2a, 2a+1
        rows.extend(range(C + 128 * a, C + 128 * (a + 1)))  # k heads 2a, 2a+1
    rows.extend(range(2 * C, 3 * C))                        # v natural
    W_perm = W[rows]
    b_perm = b[rows]

    e4 = ml_dtypes.float8_e4m3
    wT = np.ascontiguousarray(W_perm.T) * np.float32(SW)     # (C, 3C)
    wh = wT.astype(e4)
    wl = (wT - wh.astype(np.float32)).astype(e4)
    bqk = np.ascontiguousarray(b_perm[:2 * C].reshape(2 * NPAIR, 128).T)  # (128, 12)
    bvb = np.ascontiguousarray(np.broadcast_to(b_perm[2 * C:], (128, C)))
    tri = (np.arange(128)[None, :] >= np.arange(128)[:, None]).astype(np.float32)
    masks = np.ones((128, T), dtype=np.float32)
    masks[:, 0:128] = tri          # kept windows always start at the diagonal
    zeros = np.zeros((128, T - 128), dtype=np.float16)
    xT = np.asarray(x, dtype=np.float32).transpose(0, 2, 1) * np.float32(SX)  # (B, C, T)
    xhv = xT.astype(e4)
    xlv = (xT - xhv.astype(np.float32)).astype(e4)
    return xhv, xlv, wh, wl, bqk, bvb, masks, zeros


def kernel(x, W_attn, b_attn):
    if "nc" not in _CACHE:
        _CACHE["nc"] = _build()
    nc = _CACHE["nc"]

    xhv, xlv, wh, wl, bqk, bvb, masks, zeros = _prep_host(x, W_attn, b_attn)
    in_maps = [
        {"xh": xhv[c], "xl": xlv[c], "wh": wh, "wl": wl, "bqk": bqk,
         "bvb": bvb, "masks": masks, "zeros": zeros}
        for c in range(B)
    ]
    res = run_bass_kernel_spmd(nc, in_maps, list(range(B)))
    y = np.empty((B, T, C), dtype=np.float32)
    for c in range(B):
        y[c] = res.results[c]["yT"].T.astype(np.float32)
    return y


# revision 6
# speedup vs baseline: 1.1252x; 1.1252x over previous
"""Trainium2 Bass kernel for causal masked-ReLU attention (no softmax).

Reference computation (B=8, T=1024, C=768, n_head=12, hd=64):
    qkv = x @ W_attn.T + b_attn
    q, k, v = split(qkv); per-head: att = relu(mask_causal(q k^T / sqrt(hd)))
    y = att @ v, heads re-merged -> (B, T, C)

Sharding: one batch element per NeuronCore (8 cores). Each core computes the
QKV projection and all 12 heads' attention for its batch element.

Layout strategy (per core):
  - Host passes x[b].T (C, T) and W.T (C, 3C) so the contraction dim C lands
    on SBUF partitions with unit-stride DMA (no on-chip transposes).
  - W rows are pre-permuted on host into [q-pair0, k-pair0, q-pair1, ...] so
    q.T / k.T of head h live at the same partition offset (h%2)*64 of their
    M-tiles; matmul operands then share a base partition.
  - q weights/bias are pre-scaled by 1/sqrt(hd) on host.
  - QKV projection runs in fp8 (e4m3) DoubleRow perf mode: 256-deep
    contraction per pass at 0.5 cycles/row = 4x fp16 PE throughput. Operands
    are split into hi+lo fp8 digits (x = xh + xl, W = wh + wl, both
    pre-scaled into e4m3's normal range) and three digit products
    xh*wh + xl*wh + xh*wl accumulate in the same fp32 PSUM group; the
    dropped xl*wl term is ~1e-4 relative. The 2^13 operand scaling is
    removed at eviction (activation scale / tensor_scalar multiply), where
    the bias is also added. Net error ~1.2e-3, PE cost 0.75x of fp16.
  - att is computed transposed (att.T = k @ q.T, layout [T_k, T_q]) so the AV
    matmul (y.T = v.T @ att.T) streams att.T directly with v as stationary.
    QK/AV run in fp16 (2-digit fp8 would double the eviction traffic; 1-digit
    fp8 error ~2.7% exceeds the tolerance).
  - Causal structure at 256-col granularity: fully-masked windows are never
    computed; att.T below-diagonal regions are zeroed once and never written.
  - Input DMAs are chained (~6 in flight) ordered so the v-projection's
    hi/lo k-pair tiles land first, then the q/k weight tiles.
  - QK evictions and AV accumulation groups both run ascending, so att
    tiles are read and released in the same order the next head rewrites
    them, and the QK eviction stream ends on the cheap diagonal tiles.
  - Eviction work (masked-ReLU, bias adds, y copies) is balanced across
    the ACT and DVE engines, which sit just under the PE's per-head time.
  - A few dummy matmuls during the initial DMA wait keep the PE's HAM
    activity window warm so real matmuls start at the full 2.4 GHz clock.
  - Output is written as y.T (C, T) in fp16; host transposes and upcasts.
"""

import numpy as np

import sys
for _p in ("/opt/trn_rl_repo", "/root/.axon_site", "/root/.axon_site/_ro/trn_rl_repo",
           "/root/.axon_site/_ro/pypackages"):
    if _p not in sys.path:
        sys.path.append(_p)

import ml_dtypes

import concourse.bacc as bacc
import concourse.mybir as mybir
from concourse.alu_op_type import AluOpType
from concourse.tile import TileContext
from concourse.tile_rust import add_dep_helper
from concourse.bass_utils import run_bass_kernel_spmd

B, T, C = 8, 1024, 768
NH, HD = 12, 64
C3 = 3 * C            # 2304
KT = C // 128         # 6  contraction tiles of the projection
NP = KT // 2          # 3  contraction pairs (DoubleRow)
TT = T // 128         # 8  tiles of the sequence dim
NPAIR = NH // 2       # 6  head pairs
NW = T // 256         # 4  256-wide attention windows
F32 = mybir.dt.float32
F16 = mybir.dt.float16
F8 = mybir.dt.float8e4
AF = mybir.ActivationFunctionType
DR = mybir.MatmulPerfMode.DoubleRow

SX = 16.0             # x pre-scale (keeps x-lo digits in e4m3 normal range)
SW = 512.0            # W pre-scale
DESCALE = 1.0 / (SX * SW)

WARM_MMS = 8

_CACHE = {}


def _build():
    nc = bacc.Bacc("TRN2", target_bir_lowering=False, debug=False, num_devices=8)

    # host packs all fp8 operands partition-major ([128, ...] with each
    # partition's six k-tiles contiguous) so one DMA moves a whole k-pair
    # (or tensor) with 2KB+ descriptor runs: the HWDGE's fixed ~625ns cost
    # per DMA dominates the input stream otherwise
    xh = nc.dram_tensor("xh", [128, KT * T], F8, kind="ExternalInput").ap()
    xl = nc.dram_tensor("xl", [128, KT * T], F8, kind="ExternalInput").ap()
    wvh = nc.dram_tensor("wvh", [128, KT * C], F8, kind="ExternalInput").ap()
    wvl = nc.dram_tensor("wvl", [128, KT * C], F8, kind="ExternalInput").ap()
    wqh = nc.dram_tensor("wqh", [128, KT * 2 * C], F8, kind="ExternalInput").ap()
    wql = nc.dram_tensor("wql", [128, KT * 2 * C], F8, kind="ExternalInput").ap()
    bqk = nc.dram_tensor("bqk", [128, 2 * NPAIR], F32, kind="ExternalInput").ap()
    bvb = nc.dram_tensor("bvb", [128, C], F32, kind="ExternalInput").ap()
    # masks = [tri(128) | ones(896)]: the kept region of att.T tile tk always
    # starts with the triangular diagonal block, so masks[:, :width] is the
    # relu-mask for any kept window
    masks = nc.dram_tensor("masks", [128, T], F32, kind="ExternalInput").ap()
    yT = nc.dram_tensor("yT", [C, T], F16, kind="ExternalOutput").ap()

    with TileContext(nc) as tc:
        with (
            tc.tile_pool(name="persist", bufs=1) as pp,
            tc.tile_pool(name="psum_y", bufs=2, space="PSUM") as ps_y,
        ):
            masks_sb = pp.tile([128, T], F32, name="masks_sb")
            bqk_sb = pp.tile([128, 2 * NPAIR], F32, name="bqk_sb")
            bvb_sb = pp.tile([128, C], F32, name="bvb_sb")
            qkT = [pp.tile([128, T], F16, name=f"qkT{m}") for m in range(2 * NPAIR)]
            v_sb = [pp.tile([128, C], F16, name=f"v{t}") for t in range(TT)]
            attsets = [[pp.tile([128, T], F16, name=f"att{s}_{t}")
                        for t in range(TT)] for s in range(2)]

            # ---------- Phase 1: QKV projection (fp8 DoubleRow, 3 digit
            # products xh*wh + xl*wh + xh*wl into one PSUM group) ----------
            with (
                tc.tile_pool(name="io", bufs=1) as iop,
                tc.tile_pool(name="psum_proj", bufs=6, space="PSUM") as ps_proj,
            ):
                # dim1 indexes the six 128-deep contraction sub-tiles; a
                # DoubleRow matmul consumes a [:, 2p:2p+2, :] pair per pass
                xh_sb = iop.tile([128, KT, T], F8, name="xh_sb")
                xl_sb = iop.tile([128, KT, T], F8, name="xl_sb")
                wv_h = iop.tile([128, KT, C], F8, name="wv_h")
                wv_l = iop.tile([128, KT, C], F8, name="wv_l")
                wq_h = iop.tile([128, KT, 2 * C], F8, name="wq_h")
                wq_l = iop.tile([128, KT, 2 * C], F8, name="wq_l")

                # input DMAs: per k-pair, x hi/lo + the v-slice of W hi/lo
                # first (v windows run first and consume digits in this
                # order), then the q/k weights, then the phase-2 masks
                smalls = [(bvb_sb, bvb), (bqk_sb, bqk)]
                for p in range(NP):
                    sl2 = slice(2 * T * p, 2 * T * (p + 1))
                    slv = slice(2 * C * p, 2 * C * (p + 1))
                    nc.sync.dma_start(out=xh_sb[:, 2 * p:2 * p + 2, :],
                                      in_=xh[:, sl2])
                    nc.sync.dma_start(out=wv_h[:, 2 * p:2 * p + 2, :],
                                      in_=wvh[:, slv])
                    nc.sync.dma_start(out=xl_sb[:, 2 * p:2 * p + 2, :],
                                      in_=xl[:, sl2])
                    nc.sync.dma_start(out=wv_l[:, 2 * p:2 * p + 2, :],
                                      in_=wvl[:, slv])
                    if smalls:
                        dst, src_ = smalls.pop(0)
                        nc.sync.dma_start(out=dst[:], in_=src_[:])
                for p in range(NP):
                    slq = slice(4 * C * p, 4 * C * (p + 1))
                    nc.sync.dma_start(out=wq_h[:, 2 * p:2 * p + 2, :],
                                      in_=wqh[:, slq])
                    nc.sync.dma_start(out=wq_l[:, 2 * p:2 * p + 2, :],
                                      in_=wql[:, slq])
                nc.sync.dma_start(out=masks_sb[:], in_=masks[:])

                # PE warmup: dummy matmuls on a never-written scratch tile
                # during the initial DMA wait; keeps the HAM activity window
                # busy so the real matmuls start at full clock. Results (and
                # operand garbage) are discarded.
                scratch = iop.tile([128, 512], F16, name="warm_src")
                nc.vector.memset(scratch[:], 0.0)
                warm = ps_proj.tile([128, 512], F32, name="ps_warm", tag="ps_proj")
                for _ in range(WARM_MMS):
                    nc.tensor.matmul(warm[:], scratch[:, :128], scratch[:],
                                     start=True, stop=True)

                # each group = one [128, 512] PSUM tile (one full bank / zero
                # region) holding one or two 256-wide DoubleRow chunks.
                # ("v", t, n0, width) / ("qk", m, q0, width)
                groups = []
                for t in range(TT):
                    groups.append(("v", t, 0, 512))
                    groups.append(("v", t, 512, 256))
                for m in range(2 * NPAIR):
                    for q0 in (0, 512):
                        groups.append(("qk", m, q0, 512))

                # windows of 5 psum tiles; k-pair-major, digit-product-minor
                # within the window so the PE's consumption order matches the
                # per-pair DMA arrival order (xh, wvh, xl, wvl per pair).
                for w0 in range(0, len(groups), 5):
                    window = groups[w0:w0 + 5]
                    tiles = [ps_proj.tile([128, 512], F32, name="ps_proj",
                                          tag="ps_proj") for _ in window]
                    nmm = {id(ps): 0 for ps in tiles}
                    total = {id(ps): 9 * (g[3] // 256)
                             for g, ps in zip(window, tiles)}
                    for p in range(NP):
                        pr = slice(2 * p, 2 * p + 2)
                        for term in range(3):
                            xa = (xh_sb, xl_sb, xh_sb)[term]
                            wva = (wv_h, wv_h, wv_l)[term]
                            wqa = (wq_h, wq_h, wq_l)[term]
                            for g, ps in zip(window, tiles):
                                kind, i, o0, wd = g
                                for c0 in range(0, wd, 256):
                                    n = nmm[id(ps)]
                                    nmm[id(ps)] = n + 1
                                    st = n == 0
                                    sp = n == total[id(ps)] - 1
                                    if kind == "v":
                                        nc.tensor.matmul(
                                            ps[:, c0:c0 + 256],
                                            xa[:, pr, 128 * i:128 * (i + 1)],
                                            wva[:, pr, o0 + c0:o0 + c0 + 256],
                                            start=st, stop=sp, perf_mode=DR,
                                        )
                                    else:
                                        nc.tensor.matmul(
                                            ps[:, c0:c0 + 256],
                                            wqa[:, pr, 128 * i:128 * (i + 1)],
                                            xa[:, pr, o0 + c0:o0 + c0 + 256],
                                            start=st, stop=sp, perf_mode=DR,
                                        )
                    for g, ps in zip(window, tiles):
                        kind, i, o0, wd = g
                        if kind == "v":
                            nc.vector.scalar_tensor_tensor(
                                v_sb[i][:, o0:o0 + wd], ps[:, :wd], DESCALE,
                                bvb_sb[:, o0:o0 + wd],
                                AluOpType.mult, AluOpType.add,
                            )
                        elif i % 2 == 0:
                            nc.scalar.activation(
                                qkT[i][:, o0:o0 + wd], ps[:, :wd],
                                AF.Identity, bias=bqk_sb[:, i:i + 1],
                                scale=DESCALE,
                            )
                        else:
                            nc.vector.tensor_scalar(
                                qkT[i][:, o0:o0 + wd], ps[:, :wd],
                                DESCALE, bqk_sb[:, i:i + 1],
                                AluOpType.mult, AluOpType.add,
                            )

            # ---------- Phase 2: attention, head by head ----------
            with (
                tc.tile_pool(name="psum_att", bufs=6, space="PSUM") as ps_att,
                tc.tile_pool(name="yout", bufs=2) as yop,
            ):
                for a in range(NPAIR):
                    y_pair = yop.tile([128, T], F16, name="y_pair", tag="y_pair")
                    for phase in range(2):   # 0 = QK both heads, 1 = AV both heads
                      for r in range(2):
                        h = 2 * a + r
                        att = attsets[r]
                        qh = qkT[2 * a][64 * r:64 * (r + 1), :]
                        kh = qkT[2 * a + 1][64 * r:64 * (r + 1), :]
                        if phase == 0:
                          # QK^T -> att.T, tk ascending, single-bank psum tiles
                          # for fine-grained slot recycling. Ascending order
                          # ends on the cheap evictions (tk6/7 have no relu),
                          # so AV never waits behind an eviction backlog.
                          for tk in range(TT):
                              k0 = 128 * tk
                              q0d = 256 * (tk // 2)       # start of diag window
                              # piece 1: [k0, 512) if the diag sits left of 512
                              if q0d < 512:
                                  tag = "ps_y" if (h == 0 and tk <= 1) else "ps_att"
                                  pool = ps_y if tag == "ps_y" else ps_att
                                  ps = pool.tile([128, 512], F32,
                                                 name="ps_qk", tag=tag)
                                  pw = 512 - q0d
                                  nc.tensor.matmul(
                                      ps[:, k0 - q0d:pw], kh[:, k0:k0 + 128],
                                      qh[:, k0:512], start=True, stop=True,
                                  )
                                  # whole piece in one DVE op: relu * [tri|1..]
                                  nc.vector.scalar_tensor_tensor(
                                      att[tk][:, k0:512],
                                      ps[:, k0 - q0d:pw],
                                      0.0, masks_sb[:, :512 - k0],
                                      AluOpType.max, AluOpType.mult,
                                  )
                                  # piece 2: the full [512, 1024) half
                                  ps = pool.tile([128, 512], F32,
                                                 name="ps_qk", tag=tag)
                                  nc.tensor.matmul(
                                      ps[:], kh[:, k0:k0 + 128], qh[:, 512:T],
                                      start=True, stop=True,
                                  )
                                  nc.scalar.activation(att[tk][:, 512:T], ps[:],
                                                       AF.Relu)
                              else:
                                  # single piece [k0, 1024)
                                  ps = ps_att.tile([128, 512], F32,
                                                   name="ps_qk", tag="ps_att")
                                  pw = T - q0d
                                  nc.tensor.matmul(
                                      ps[:, k0 - q0d:pw], kh[:, k0:k0 + 128],
                                      qh[:, k0:T], start=True, stop=True,
                                  )
                                  nc.vector.scalar_tensor_tensor(
                                      att[tk][:, k0:T],
                                      ps[:, k0 - q0d:pw],
                                      0.0, masks_sb[:, :T - k0],
                                      AluOpType.max, AluOpType.mult,
                                  )

                        if phase == 1:
                          # AV: y.T = v.T @ att.T, groups j ascending, paired
                          # into one [64, 512] psum tile per half; att tiles are
                          # read and released in the order the next head's QK
                          # rewrites them
                          jhis = (3, 1) if h == NH - 1 else (1, 3)
                          for jhi in jhis:
                              ps2 = ps_y.tile([64, 512], F32, name="ps_av",
                                              tag="ps_y")
                              for j in (jhi - 1, jhi):
                                  q0 = 256 * j
                                  c0 = 256 * (j - (jhi - 1))
                                  ntk = min(TT, 2 * j + 2)
                                  for tk in range(ntk):
                                      lo = 128 if tk == 2 * j + 1 else 0
                                      nc.tensor.matmul(
                                          ps2[:, c0 + lo:c0 + 256],
                                          v_sb[tk][:, 64 * h:64 * (h + 1)],
                                          att[tk][:, q0 + lo:q0 + 256],
                                          start=(tk == 0), stop=(tk == ntk - 1),
                                      )
                              nc.scalar.copy(
                                  y_pair[64 * r:64 * (r + 1),
                                         256 * (jhi - 1):256 * (jhi + 1)],
                                  ps2[:],
                              )
                              nc.sync.dma_start(
                                  out=yT[128 * a + 64 * r:128 * a + 64 * (r + 1),
                                         256 * (jhi - 1):256 * (jhi + 1)],
                                  in_=y_pair[64 * r:64 * (r + 1),
                                             256 * (jhi - 1):256 * (jhi + 1)])

    nc.compile()
    return nc

def _prep_host(x, W_attn, b_attn):
    s = 1.0 / np.sqrt(np.float32(HD))
    W = np.asarray(W_attn, dtype=np.float32).copy()
    b = np.asarray(b_attn, dtype=np.float32).copy()
    W[:C] *= s
    b[:C] *= s
    # interleave q/k head pairs: [q-pair0, k-pair0, q-pair1, k-pair1, ...], v natural
    rows = []
    for a in range(NPAIR):
        rows.extend(range(128 * a, 128 * (a + 1)))          # q heads 2a, 2a+1
        rows.extend(range(C + 128 * a, C + 128 * (a + 1)))  # k heads 2a, 2a+1
    rows.extend(range(2 * C, 3 * C))                        # v natural
    W_perm = W[rows]
    b_perm = b[rows]

    e4 = ml_dtypes.float8_e4m3

    def pack(mat):
        # (C, N) -> partition-major (128, KT*N): each partition's six
        # contraction k-tiles contiguous, k-pair-major
        Cr, N = mat.shape
        return np.ascontiguousarray(
            mat.reshape(KT, 128, N).transpose(1, 0, 2).reshape(128, KT * N))

    def split8(mat):
        hi = mat.astype(e4)
        lo = (mat - hi.astype(np.float32)).astype(e4)
        return hi, lo

    wT = np.ascontiguousarray(W_perm.T) * np.float32(SW)     # (C, 3C)
    wqh, wql = split8(pack(wT[:, :2 * C]))
    wvh, wvl = split8(pack(wT[:, 2 * C:]))
    bqk = np.ascontiguousarray(b_perm[:2 * C].reshape(2 * NPAIR, 128).T)  # (128, 12)
    bvb = np.ascontiguousarray(np.broadcast_to(b_perm[2 * C:], (128, C)))
    tri = (np.arange(128)[None, :] >= np.arange(128)[:, None]).astype(np.float32)
    masks = np.ones((128, T), dtype=np.float32)
    masks[:, 0:128] = tri          # kept windows always start at the diagonal
    xT = np.asarray(x, dtype=np.float32).transpose(0, 2, 1) * np.float32(SX)  # (B, C, T)
    xhv = np.stack([pack(xT[c]) for c in range(B)])
    xhv, xlv = split8(xhv)
    return xhv, xlv, wqh, wql, wvh, wvl, bqk, bvb, masks


def kernel(x, W_attn, b_attn):
    if "nc" not in _CACHE:
        _CACHE["nc"] = _build()
    nc = _CACHE["nc"]

    xhv, xlv, wqh, wql, wvh, wvl, bqk, bvb, masks = _prep_host(x, W_attn, b_attn)
    in_maps = [
        {"xh": xhv[c], "xl": xlv[c], "wqh": wqh, "wql": wql, "wvh": wvh,
         "wvl": wvl, "bqk": bqk, "bvb": bvb, "masks": masks}
        for c in range(B)
    ]
    res = run_bass_kernel_spmd(nc, in_maps, list(range(B)))
    y = np.empty((B, T, C), dtype=np.float32)
    for c in range(B):
        y[c] = res.results[c]["yT"].T.astype(np.float32)
    return y


# revision 57
# speedup vs baseline: 1.1596x; 1.0306x over previous
"""Trainium2 Bass kernel for causal masked-ReLU attention (no softmax).

Reference computation (B=8, T=1024, C=768, n_head=12, hd=64):
    qkv = x @ W_attn.T + b_attn
    q, k, v = split(qkv); per-head: att = relu(mask_causal(q k^T / sqrt(hd)))
    y = att @ v, heads re-merged -> (B, T, C)

Sharding: one batch element per NeuronCore (8 cores). Each core computes the
QKV projection and all 12 heads' attention for its batch element.

Layout strategy (per core):
  - Host passes x[b].T (C, T) and W.T (C, 3C) so the contraction dim C lands
    on SBUF partitions with unit-stride DMA (no on-chip transposes).
  - W rows are pre-permuted on host into [q-pair0, k-pair0, q-pair1, ...] so
    q.T / k.T of head h live at the same partition offset (h%2)*64 of their
    M-tiles; matmul operands then share a base partition.
  - q weights/bias are pre-scaled by 1/sqrt(hd) on host.
  - QKV projection runs in fp8 (e4m3) DoubleRow perf mode: 256-deep
    contraction per pass at 0.5 cycles/row = 4x fp16 PE throughput. Operands
    are split into hi+lo fp8 digits (x = xh + xl, W = wh + wl, both
    pre-scaled into e4m3's normal range) and three digit products
    xh*wh + xl*wh + xh*wl accumulate in the same fp32 PSUM group; the
    dropped xl*wl term is ~1e-4 relative. The 2^13 operand scaling is
    removed at eviction (activation scale / tensor_scalar multiply), where
    the bias is also added. Net error ~1.2e-3, PE cost 0.75x of fp16.
  - att is computed transposed (att.T = k @ q.T, layout [T_k, T_q]) so the AV
    matmul (y.T = v.T @ att.T) streams att.T directly with v as stationary.
    QK/AV run in fp16 (2-digit fp8 would double the eviction traffic; 1-digit
    fp8 error ~2.7% exceeds the tolerance).
  - Causal structure at 128-col granularity: fully-masked regions are never
    computed or read (AV's accumulation prefix never touches below-diagonal
    att, so no zero-fill is needed).
  - All fp8 operands are packed partition-major on the host so each k-pair
    (or whole digit tensor) moves in ONE DMA with 2KB+ descriptor runs: the
    HWDGE's fixed ~625ns per-DMA cost otherwise serializes the input stream
    (19 input DMAs total, ordered v-projection digits first).
  - One pool scope spans both phases (a pool close = all-engine barrier);
    projection windows borrow the spare psum ring for 8 tiles in flight,
    except the last two windows, whose spare slots phase 2's first QK
    pieces pick up barrier-free.
  - QK evictions and AV accumulation groups both run ascending, so att
    tiles are read and released in the same order the next head rewrites
    them, and the QK eviction stream ends on the cheap diagonal tiles.
  - Eviction work (masked-ReLU, bias adds, y copies) is balanced across
    the ACT and DVE engines, which sit just under the PE's per-head time;
    the final head's last AV group is split into [64,256] quarters with
    copies on both engines so the closing copy+DMA+sem chain is minimal.
  - Output is written as y.T (C, T) in fp16; host transposes and upcasts.
"""

import numpy as np

import sys
for _p in ("/opt/trn_rl_repo", "/root/.axon_site", "/root/.axon_site/_ro/trn_rl_repo",
           "/root/.axon_site/_ro/pypackages"):
    if _p not in sys.path:
        sys.path.append(_p)

import ml_dtypes

import concourse.bacc as bacc
import concourse.mybir as mybir
from concourse.alu_op_type import AluOpType
from concourse.tile import TileContext
from concourse.tile_rust import add_dep_helper
from concourse.bass_utils import run_bass_kernel_spmd

B, T, C = 8, 1024, 768
NH, HD = 12, 64
C3 = 3 * C            # 2304
KT = C // 128         # 6  contraction tiles of the projection
NP = KT // 2          # 3  contraction pairs (DoubleRow)
TT = T // 128         # 8  tiles of the sequence dim
NPAIR = NH // 2       # 6  head pairs
NW = T // 256         # 4  256-wide attention windows
F32 = mybir.dt.float32
F16 = mybir.dt.float16
F8 = mybir.dt.float8e4
AF = mybir.ActivationFunctionType
DR = mybir.MatmulPerfMode.DoubleRow

SX = 16.0             # x pre-scale (keeps x-lo digits in e4m3 normal range)
SW = 512.0            # W pre-scale
DESCALE = 1.0 / (SX * SW)

WARM_MMS = 0

_CACHE = {}


def _build():
    nc = bacc.Bacc("TRN2", target_bir_lowering=False, debug=False, num_devices=8)

    # host packs all fp8 operands partition-major ([128, ...] with each
    # partition's six k-tiles contiguous) so one DMA moves a whole k-pair
    # (or tensor) with 2KB+ descriptor runs: the HWDGE's fixed ~625ns cost
    # per DMA dominates the input stream otherwise
    xh = nc.dram_tensor("xh", [128, KT * T], F8, kind="ExternalInput").ap()
    xl = nc.dram_tensor("xl", [128, KT * T], F8, kind="ExternalInput").ap()
    wvh = nc.dram_tensor("wvh", [128, KT * C], F8, kind="ExternalInput").ap()
    wvl = nc.dram_tensor("wvl", [128, KT * C], F8, kind="ExternalInput").ap()
    wqh = nc.dram_tensor("wqh", [128, KT, 2 * C], F8, kind="ExternalInput").ap()
    wql = nc.dram_tensor("wql", [128, KT, 2 * C], F8, kind="ExternalInput").ap()
    bqk = nc.dram_tensor("bqk", [128, 2 * NPAIR], F32, kind="ExternalInput").ap()
    bvb = nc.dram_tensor("bvb", [128, C], F16, kind="ExternalInput").ap()
    # masks = [tri(128) | ones(896)]: the kept region of att.T tile tk always
    # starts with the triangular diagonal block, so masks[:, :width] is the
    # relu-mask for any kept window
    masks = nc.dram_tensor("masks", [128, T], F32, kind="ExternalInput").ap()
    yT = nc.dram_tensor("yT", [C, T], F16, kind="ExternalOutput").ap()

    with TileContext(nc) as tc:
        with (
            tc.tile_pool(name="persist", bufs=1) as pp,
        ):
            masks_sb = pp.tile([128, T], F32, name="masks_sb")
            bqk_sb = pp.tile([128, 2 * NPAIR], F32, name="bqk_sb")
            bvb_sb = pp.tile([128, C], F16, name="bvb_sb")
            qkT = [pp.tile([128, T], F16, name=f"qkT{m}") for m in range(2 * NPAIR)]
            v_sb = [pp.tile([128, C], F16, name=f"v{t}") for t in range(TT)]
            attsets = [[pp.tile([128, T], F16, name=f"att{s}_{t}")
                        for t in range(TT)] for s in range(2)]

            # ---------- Phase 1: QKV projection (fp8 DoubleRow, 3 digit
            # products xh*wh + xl*wh + xh*wl into one PSUM group) ----------
            # The io/psum pools deliberately stay open across both phases:
            # closing a pool inserts an all-engine barrier that idles the PE
            # for ~1.5us at the phase boundary. Phase 2's QK psum tiles come
            # from the same rotation, so the first heads naturally pipeline
            # behind the last projection windows.
            from contextlib import ExitStack
            with ExitStack() as stack:
                iop = stack.enter_context(tc.tile_pool(name="io", bufs=1))
                # one 8-slot ring covering all PSUM use in both phases: more
                # projection tiles in flight (the stall there is tiles-in-
                # flight-bound while the input DMAs stream), and no pool
                # barrier between phases
                ps_proj = stack.enter_context(
                    tc.tile_pool(name="psum_proj", bufs=6, space="PSUM"))
                ps_y = stack.enter_context(
                    tc.tile_pool(name="psum_y", bufs=2, space="PSUM"))
                yop = stack.enter_context(tc.tile_pool(name="yout", bufs=2))
                # dim1 indexes the six 128-deep contraction sub-tiles; a
                # DoubleRow matmul consumes a [:, 2p:2p+2, :] pair per pass
                xh_sb = iop.tile([128, KT, T], F8, name="xh_sb")
                xl_sb = iop.tile([128, KT, T], F8, name="xl_sb")
                wv_h = iop.tile([128, KT, C], F8, name="wv_h")
                wv_l = iop.tile([128, KT, C], F8, name="wv_l")
                wq_h = iop.tile([128, KT, 2 * C], F8, name="wq_h")
                wq_l = iop.tile([128, KT, 2 * C], F8, name="wq_l")

                # input DMAs: per k-pair, x hi/lo + the v-slice of W hi/lo
                # first (v windows run first and consume digits in this
                # order), then the q/k weights, then the phase-2 masks.
                # The bias tensors slot in where the first evictions need
                # them without delaying the pair stream's front.
                for p in range(NP):
                    sl2 = slice(2 * T * p, 2 * T * (p + 1))
                    slv = slice(2 * C * p, 2 * C * (p + 1))
                    nc.sync.dma_start(out=xh_sb[:, 2 * p:2 * p + 2, :],
                                      in_=xh[:, sl2])
                    nc.sync.dma_start(out=wv_h[:, 2 * p:2 * p + 2, :],
                                      in_=wvh[:, slv])
                    nc.sync.dma_start(out=xl_sb[:, 2 * p:2 * p + 2, :],
                                      in_=xl[:, sl2])
                    nc.sync.dma_start(out=wv_l[:, 2 * p:2 * p + 2, :],
                                      in_=wvl[:, slv])
                    if p == 1:
                        nc.sync.dma_start(out=bvb_sb[:], in_=bvb[:])
                    elif p == 2:
                        nc.sync.dma_start(out=bqk_sb[:], in_=bqk[:])
                # pair 0's q/k weights ship in m0-m3 / m4-m11 halves so
                # the first qk windows start ~1us earlier
                pr0 = slice(0, 2)
                nc.sync.dma_start(out=wq_h[:, pr0, :512], in_=wqh[:, pr0, :512])
                nc.sync.dma_start(out=wq_l[:, pr0, :512], in_=wql[:, pr0, :512])
                nc.sync.dma_start(out=wq_h[:, pr0, 512:], in_=wqh[:, pr0, 512:])
                nc.sync.dma_start(out=wq_l[:, pr0, 512:], in_=wql[:, pr0, 512:])
                for p in range(1, NP):
                    prp = slice(2 * p, 2 * p + 2)
                    nc.sync.dma_start(out=wq_h[:, prp, :], in_=wqh[:, prp, :])
                    nc.sync.dma_start(out=wq_l[:, prp, :], in_=wql[:, prp, :])
                nc.sync.dma_start(out=masks_sb[:], in_=masks[:])

                # PE warmup: dummy matmuls on a never-written scratch tile
                # during the initial DMA wait; keeps the HAM activity window
                # busy so the real matmuls start at full clock. Results (and
                # operand garbage) are discarded.
                if WARM_MMS:
                    scratch = iop.tile([128, 512], F16, name="warm_src")
                    nc.vector.memset(scratch[:], 0.0)
                    warm = ps_proj.tile([128, 512], F32, name="ps_warm",
                                        tag="ps_proj")
                    for _ in range(WARM_MMS):
                        nc.tensor.matmul(warm[:], scratch[:, :128], scratch[:],
                                         start=True, stop=True)

                # each group = one [128, 512] PSUM tile (one full bank / zero
                # region) holding one or two 256-wide DoubleRow chunks.
                # ("v", t, n0, width) / ("qk", m, q0, width)
                groups = []
                for t in range(TT):
                    groups.append(("v", t, 0, 512))
                    groups.append(("v", t, 512, 256))
                for m in range(2 * NPAIR):
                    for q0 in (0, 512):
                        groups.append(("qk", m, q0, 512))

                # windows of 4 psum tiles; k-pair-major, digit-product-minor
                # within the window so the PE's consumption order matches the
                # per-pair DMA arrival order (xh, wvh, xl, wvl per pair).
                nwin = (len(groups) + 3) // 4
                for wi, w0 in enumerate(range(0, len(groups), 4)):
                    window = groups[w0:w0 + 4]
                    # borrow the ps_y ring for extra tiles in flight, except
                    # in the last two windows: phase 2's first QK pieces then
                    # find those slots free at the phase boundary
                    borrow = wi < nwin - 2
                    tiles = [(ps_y if (borrow and gi >= 3) else ps_proj).tile(
                                 [128, 512], F32, name="ps_proj",
                                 tag="ps_y" if (borrow and gi >= 3) else "ps_proj")
                             for gi in range(len(window))]
                    nmm = {id(ps): 0 for ps in tiles}
                    total = {id(ps): 9 * (g[3] // 256)
                             for g, ps in zip(window, tiles)}
                    for p in range(NP):
                        pr = slice(2 * p, 2 * p + 2)
                        for term in range(3):
                            xa = (xh_sb, xl_sb, xh_sb)[term]
                            wva = (wv_h, wv_h, wv_l)[term]
                            wqa = (wq_h, wq_h, wq_l)[term]
                            for g, ps in zip(window, tiles):
                                kind, i, o0, wd = g
                                for c0 in range(0, wd, 256):
                                    n = nmm[id(ps)]
                                    nmm[id(ps)] = n + 1
                                    st = n == 0
                                    sp = n == total[id(ps)] - 1
                                    if kind == "v":
                                        nc.tensor.matmul(
                                            ps[:, c0:c0 + 256],
                                            xa[:, pr, 128 * i:128 * (i + 1)],
                                            wva[:, pr, o0 + c0:o0 + c0 + 256],
                                            start=st, stop=sp, perf_mode=DR,
                                        )
                                    else:
                                        nc.tensor.matmul(
                                            ps[:, c0:c0 + 256],
                                            wqa[:, pr, 128 * i:128 * (i + 1)],
                                            xa[:, pr, o0 + c0:o0 + c0 + 256],
                                            start=st, stop=sp, perf_mode=DR,
                                        )
                    for g, ps in zip(window, tiles):
                        kind, i, o0, wd = g
                        if kind == "v":
                            nc.vector.scalar_tensor_tensor(
                                v_sb[i][:, o0:o0 + wd], ps[:, :wd], DESCALE,
                                bvb_sb[:, o0:o0 + wd],
                                AluOpType.mult, AluOpType.add,
                            )
                        elif i % 2 == 0:
                            nc.scalar.activation(
                                qkT[i][:, o0:o0 + wd], ps[:, :wd],
                                AF.Identity, bias=bqk_sb[:, i:i + 1],
                                scale=DESCALE,
                            )
                        else:
                            nc.vector.tensor_scalar(
                                qkT[i][:, o0:o0 + wd], ps[:, :wd],
                                DESCALE, bqk_sb[:, i:i + 1],
                                AluOpType.mult, AluOpType.add,
                            )

            # ---------- Phase 2: attention, head by head ----------
            # (still inside the io/ps_proj pool scope — no phase barrier)
            if True:
                ps_att = ps_proj
                for a in range(NPAIR):
                    y_pair = yop.tile([128, T], F16, name="y_pair", tag="y_pair")
                    for phase in range(2):   # 0 = QK both heads, 1 = AV both heads
                      for r in range(2):
                        h = 2 * a + r
                        att = attsets[r]
                        qh = qkT[2 * a][64 * r:64 * (r + 1), :]
                        kh = qkT[2 * a + 1][64 * r:64 * (r + 1), :]
                        if phase == 0:
                          # QK^T -> att.T, tk ascending, single-bank psum tiles
                          # for fine-grained slot recycling. Ascending order
                          # ends on the cheap evictions (tk6/7 have no relu),
                          # so AV never waits behind an eviction backlog.
                          for tk in range(TT):
                              k0 = 128 * tk
                              q0d = 256 * (tk // 2)       # start of diag window
                              # piece 1: [k0, 512) if the diag sits left of 512
                              if q0d < 512:
                                  early = h == 0 and tk <= 1
                                  pool2 = ps_y if early else ps_att
                                  ps = pool2.tile([128, 512], F32,
                                                  name="ps_qk",
                                                  tag="ps_y" if early else "ps_proj")
                                  pw = 512 - q0d
                                  nc.tensor.matmul(
                                      ps[:, k0 - q0d:pw], kh[:, k0:k0 + 128],
                                      qh[:, k0:512], start=True, stop=True,
                                  )
                                  # whole piece in one DVE op: relu * [tri|1..]
                                  nc.vector.scalar_tensor_tensor(
                                      att[tk][:, k0:512],
                                      ps[:, k0 - q0d:pw],
                                      0.0, masks_sb[:, :512 - k0],
                                      AluOpType.max, AluOpType.mult,
                                  )
                                  # piece 2: the full [512, 1024) half
                                  ps = pool2.tile([128, 512], F32,
                                                  name="ps_qk",
                                                  tag="ps_y" if early else "ps_proj")
                                  nc.tensor.matmul(
                                      ps[:], kh[:, k0:k0 + 128], qh[:, 512:T],
                                      start=True, stop=True,
                                  )
                                  nc.scalar.activation(att[tk][:, 512:T], ps[:],
                                                       AF.Relu)
                              else:
                                  # single piece [k0, 1024)
                                  ps = ps_att.tile([128, 512], F32,
                                                   name="ps_qk", tag="ps_proj")
                                  pw = T - q0d
                                  nc.tensor.matmul(
                                      ps[:, k0 - q0d:pw], kh[:, k0:k0 + 128],
                                      qh[:, k0:T], start=True, stop=True,
                                  )
                                  nc.vector.scalar_tensor_tensor(
                                      att[tk][:, k0:T],
                                      ps[:, k0 - q0d:pw],
                                      0.0, masks_sb[:, :T - k0],
                                      AluOpType.max, AluOpType.mult,
                                  )

                        if phase == 1:
                          # AV: y.T = v.T @ att.T, groups j ascending, paired
                          # into one [64, 512] psum tile per half; att tiles are
                          # read and released in the order the next head's QK
                          # rewrites them
                          jhis = (3, 1) if h == NH - 1 else (1, 3)
                          yrow = slice(128 * a + 64 * r, 128 * a + 64 * (r + 1))
                          for jhi in jhis:
                              if h == NH - 1 and jhi == 1:
                                  # final half: j=1 then j=0 as separate psum
                                  # groups with their own eviction + DMA on
                                  # alternating engines, so the post-final-
                                  # matmul quantum is one [64,256] copy and
                                  # one 512B-descriptor DMA
                                  for j in (1, 0):
                                      q0 = 256 * j
                                      ntk = 2 * j + 2
                                      ps3 = ps_y.tile([64, 256], F32,
                                                      name="ps_av", tag="ps_y")
                                      for tk in range(ntk):
                                          lo = 128 if tk == 2 * j + 1 else 0
                                          nc.tensor.matmul(
                                              ps3[:, lo:],
                                              v_sb[tk][:, 64 * h:64 * (h + 1)],
                                              att[tk][:, q0 + lo:q0 + 256],
                                              start=(tk == 0),
                                              stop=(tk == ntk - 1),
                                          )
                                      ysl3 = y_pair[64 * r:64 * (r + 1),
                                                    q0:q0 + 256]
                                      nc.vector.tensor_scalar(
                                          ysl3, ps3[:], 0.0, None,
                                          AluOpType.add)
                                  nc.sync.dma_start(
                                      out=yT[yrow, 0:512],
                                      in_=y_pair[64 * r:64 * (r + 1), 0:512])
                                  continue
                              ps2 = ps_y.tile([64, 512], F32, name="ps_av",
                                              tag="ps_y")
                              for j in (jhi - 1, jhi):
                                  q0 = 256 * j
                                  c0 = 256 * (j - (jhi - 1))
                                  ntk = min(TT, 2 * j + 2)
                                  for tk in range(ntk):
                                      lo = 128 if tk == 2 * j + 1 else 0
                                      nc.tensor.matmul(
                                          ps2[:, c0 + lo:c0 + 256],
                                          v_sb[tk][:, 64 * h:64 * (h + 1)],
                                          att[tk][:, q0 + lo:q0 + 256],
                                          start=(tk == 0), stop=(tk == ntk - 1),
                                      )
                              ysl = y_pair[64 * r:64 * (r + 1),
                                           256 * (jhi - 1):256 * (jhi + 1)]
                              if a == NPAIR - 1 and r == 0 and jhi == 3:
                                  # spread the last pair's copies over both
                                  # engines: the ACT queue otherwise backs
                                  # up right before the kernel tail
                                  nc.vector.tensor_scalar(
                                      ysl, ps2[:], 0.0, None, AluOpType.add)
                              else:
                                  nc.scalar.copy(ysl, ps2[:])
                              nc.sync.dma_start(
                                  out=yT[yrow,
                                         256 * (jhi - 1):256 * (jhi + 1)],
                                  in_=ysl)

    nc.compile()
    return nc

def _prep_host(x, W_attn, b_attn):
    s = 1.0 / np.sqrt(np.float32(HD))
    W = np.asarray(W_attn, dtype=np.float32).copy()
    b = np.asarray(b_attn, dtype=np.float32).copy()
    W[:C] *= s
    b[:C] *= s
    # interleave q/k head pairs: [q-pair0, k-pair0, q-pair1, k-pair1, ...], v natural
    rows = []
    for a in range(NPAIR):
        rows.extend(range(128 * a, 128 * (a + 1)))          # q heads 2a, 2a+1
        rows.extend(range(C + 128 * a, C + 128 * (a + 1)))  # k heads 2a, 2a+1
    rows.extend(range(2 * C, 3 * C))                        # v natural
    W_perm = W[rows]
    b_perm = b[rows]

    e4 = ml_dtypes.float8_e4m3

    def pack(mat):
        # (C, N) -> partition-major (128, KT*N): each partition's six
        # contraction k-tiles contiguous, k-pair-major
        Cr, N = mat.shape
        return np.ascontiguousarray(
            mat.reshape(KT, 128, N).transpose(1, 0, 2).reshape(128, KT * N))

    def split8(mat):
        hi = mat.astype(e4)
        lo = (mat - hi.astype(np.float32)).astype(e4)
        return hi, lo

    wT = np.ascontiguousarray(W_perm.T) * np.float32(SW)     # (C, 3C)
    wqh, wql = split8(pack(wT[:, :2 * C]).reshape(128, KT, 2 * C))
    wvh, wvl = split8(pack(wT[:, 2 * C:]))
    bqk = np.ascontiguousarray(b_perm[:2 * C].reshape(2 * NPAIR, 128).T)  # (128, 12)
    bvb = np.ascontiguousarray(
        np.broadcast_to(b_perm[2 * C:], (128, C))).astype(np.float16)
    tri = (np.arange(128)[None, :] >= np.arange(128)[:, None]).astype(np.float32)
    masks = np.ones((128, T), dtype=np.float32)
    masks[:, 0:128] = tri          # kept windows always start at the diagonal
    xT = np.asarray(x, dtype=np.float32).transpose(0, 2, 1) * np.float32(SX)  # (B, C, T)
    xhv = np.stack([pack(xT[c]) for c in range(B)])
    xhv, xlv = split8(xhv)
    return xhv, xlv, wqh, wql, wvh, wvl, bqk, bvb, masks


def kernel(x, W_attn, b_attn):
    if "nc" not in _CACHE:
        _CACHE["nc"] = _build()
    nc = _CACHE["nc"]

    xhv, xlv, wqh, wql, wvh, wvl, bqk, bvb, masks = _prep_host(x, W_attn, b_attn)
    in_maps = [
        {"xh": xhv[c], "xl": xlv[c], "wqh": wqh, "wql": wql, "wvh": wvh,
         "wvl": wvl, "bqk": bqk, "bvb": bvb, "masks": masks}
        for c in range(B)
    ]
    res = run_bass_kernel_spmd(nc, in_maps, list(range(B)))
    y = np.empty((B, T, C), dtype=np.float32)
    for c in range(B):
        y[c] = res.results[c]["yT"].T.astype(np.float32)
    return y


# revision 59
# speedup vs baseline: 1.1881x; 1.0245x over previous
"""Trainium2 Bass kernel for causal masked-ReLU attention (no softmax).

Reference computation (B=8, T=1024, C=768, n_head=12, hd=64):
    qkv = x @ W_attn.T + b_attn
    q, k, v = split(qkv); per-head: att = relu(mask_causal(q k^T / sqrt(hd)))
    y = att @ v, heads re-merged -> (B, T, C)

Sharding: one batch element per NeuronCore (8 cores). Each core computes the
QKV projection and all 12 heads' attention for its batch element.

Layout strategy (per core):
  - Host passes x[b].T (C, T) and W.T (C, 3C) so the contraction dim C lands
    on SBUF partitions with unit-stride DMA (no on-chip transposes).
  - W rows are pre-permuted on host into [q-pair0, k-pair0, q-pair1, ...] so
    q.T / k.T of head h live at the same partition offset (h%2)*64 of their
    M-tiles; matmul operands then share a base partition.
  - q weights/bias are pre-scaled by 1/sqrt(hd) on host.
  - QKV projection runs in fp8 (e4m3) DoubleRow perf mode: 256-deep
    contraction per pass at 0.5 cycles/row = 4x fp16 PE throughput. Operands
    are split into hi+lo fp8 digits (x = xh + xl, W = wh + wl, both
    pre-scaled into e4m3's normal range) and three digit products
    xh*wh + xl*wh + xh*wl accumulate in the same fp32 PSUM group; the
    dropped xl*wl term is ~1e-4 relative. The 2^13 operand scaling is
    removed at eviction (activation scale / tensor_scalar multiply), where
    the bias is also added. Net error ~1.2e-3, PE cost 0.75x of fp16.
  - att is computed transposed (att.T = k @ q.T, layout [T_k, T_q]) so the AV
    matmul (y.T = v.T @ att.T) streams att.T directly with v as stationary.
    QK/AV run in fp16 (2-digit fp8 would double the eviction traffic; 1-digit
    fp8 error ~2.7% exceeds the tolerance).
  - Causal structure at 128-col granularity: fully-masked regions are never
    computed or read (AV's accumulation prefix never touches below-diagonal
    att, so no zero-fill is needed).
  - All fp8 operands are packed partition-major on the host so each k-pair
    (or whole digit tensor) moves in ONE DMA with 2KB+ descriptor runs: the
    HWDGE's fixed ~625ns per-DMA cost otherwise serializes the input stream
    (19 input DMAs total, ordered v-projection digits first).
  - One pool scope spans both phases (a pool close = all-engine barrier);
    projection windows borrow the spare psum ring for 8 tiles in flight,
    except the last two windows, whose spare slots phase 2's first QK
    pieces pick up barrier-free.
  - QK evictions and AV accumulation groups both run ascending, so att
    tiles are read and released in the same order the next head rewrites
    them, and the QK eviction stream ends on the cheap diagonal tiles.
  - Eviction work (masked-ReLU, bias adds, y copies) is balanced across
    the ACT and DVE engines, which sit just under the PE's per-head time;
    the final head's last AV group is split into [64,256] quarters with
    copies on both engines so the closing copy+DMA+sem chain is minimal.
  - Output is written as y.T (C, T) in fp16; host transposes and upcasts.
"""

import numpy as np

import sys
for _p in ("/opt/trn_rl_repo", "/root/.axon_site", "/root/.axon_site/_ro/trn_rl_repo",
           "/root/.axon_site/_ro/pypackages"):
    if _p not in sys.path:
        sys.path.append(_p)

import ml_dtypes

import concourse.bacc as bacc
import concourse.mybir as mybir
from concourse.alu_op_type import AluOpType
from concourse.tile import TileContext
from concourse.tile_rust import add_dep_helper
from concourse.bass_utils import run_bass_kernel_spmd

B, T, C = 8, 1024, 768
NH, HD = 12, 64
C3 = 3 * C            # 2304
KT = C // 128         # 6  contraction tiles of the projection
NP = KT // 2          # 3  contraction pairs (DoubleRow)
TT = T // 128         # 8  tiles of the sequence dim
NPAIR = NH // 2       # 6  head pairs
NW = T // 256         # 4  256-wide attention windows
F32 = mybir.dt.float32
F16 = mybir.dt.float16
F8 = mybir.dt.float8e4
AF = mybir.ActivationFunctionType
DR = mybir.MatmulPerfMode.DoubleRow

SX = 16.0             # x pre-scale (keeps x-lo digits in e4m3 normal range)
SW = 512.0            # W pre-scale
DESCALE = 1.0 / (SX * SW)

WARM_MMS = 0

_CACHE = {}


def _build():
    nc = bacc.Bacc("TRN2", target_bir_lowering=False, debug=False, num_devices=8)

    # host packs all fp8 operands partition-major ([128, ...] with each
    # partition's six k-tiles contiguous) so one DMA moves a whole k-pair
    # (or tensor) with 2KB+ descriptor runs: the HWDGE's fixed ~625ns cost
    # per DMA dominates the input stream otherwise
    xh = nc.dram_tensor("xh", [128, KT * T], F8, kind="ExternalInput").ap()
    xl = nc.dram_tensor("xl", [128, KT * T], F8, kind="ExternalInput").ap()
    wvh = nc.dram_tensor("wvh", [128, KT * C], F8, kind="ExternalInput").ap()
    wvl = nc.dram_tensor("wvl", [128, KT * C], F8, kind="ExternalInput").ap()
    wqh = nc.dram_tensor("wqh", [128, KT, 2 * C], F8, kind="ExternalInput").ap()
    wql = nc.dram_tensor("wql", [128, KT, 2 * C], F8, kind="ExternalInput").ap()
    bqk = nc.dram_tensor("bqk", [128, 2 * NPAIR], F32, kind="ExternalInput").ap()
    bvb = nc.dram_tensor("bvb", [128, C], F16, kind="ExternalInput").ap()
    # masks = [tri(128) | ones(896)]: the kept region of att.T tile tk always
    # starts with the triangular diagonal block, so masks[:, :width] is the
    # relu-mask for any kept window
    masks = nc.dram_tensor("masks", [128, T], F32, kind="ExternalInput").ap()
    yT = nc.dram_tensor("yT", [C, T], F16, kind="ExternalOutput").ap()

    with TileContext(nc) as tc:
        with (
            tc.tile_pool(name="persist", bufs=1) as pp,
        ):
            masks_sb = pp.tile([128, T], F32, name="masks_sb")
            bqk_sb = pp.tile([128, 2 * NPAIR], F32, name="bqk_sb")
            bvb_sb = pp.tile([128, C], F16, name="bvb_sb")
            qkT = [pp.tile([128, T], F16, name=f"qkT{m}") for m in range(2 * NPAIR)]
            v_sb = [pp.tile([128, C], F16, name=f"v{t}") for t in range(TT)]
            # att tiles 0-3 fp16; tiles 4-7 live as fp8 DoubleRow pair-tiles
            # (dim1 = tile parity). Their ~11% variance share of y keeps the
            # 1-digit fp8 error contribution ~0.9%.
            att16 = [[pp.tile([128, T], F16, name=f"att{s}_{t}")
                      for t in range(4)] for s in range(2)]
            attp = [[pp.tile([128, 2, T], F8, name=f"attp{s}_{pb}")
                     for pb in range(2)] for s in range(2)]
            # v8: on-chip hi/lo fp8 digits of v tiles 4-7, pair-packed
            v8h = [pp.tile([128, 2, C], F8, name=f"v8h{pb}") for pb in range(2)]
            v8l = [pp.tile([128, 2, C], F8, name=f"v8l{pb}") for pb in range(2)]
            attsets = [att16[s] + [attp[s][pb][:, j2, :]
                                   for pb in range(2) for j2 in range(2)]
                       for s in range(2)]

            # ---------- Phase 1: QKV projection (fp8 DoubleRow, 3 digit
            # products xh*wh + xl*wh + xh*wl into one PSUM group) ----------
            # The io/psum pools deliberately stay open across both phases:
            # closing a pool inserts an all-engine barrier that idles the PE
            # for ~1.5us at the phase boundary. Phase 2's QK psum tiles come
            # from the same rotation, so the first heads naturally pipeline
            # behind the last projection windows.
            from contextlib import ExitStack
            with ExitStack() as stack:
                iop = stack.enter_context(tc.tile_pool(name="io", bufs=1))
                # one 8-slot ring covering all PSUM use in both phases: more
                # projection tiles in flight (the stall there is tiles-in-
                # flight-bound while the input DMAs stream), and no pool
                # barrier between phases
                ps_proj = stack.enter_context(
                    tc.tile_pool(name="psum_proj", bufs=6, space="PSUM"))
                ps_y = stack.enter_context(
                    tc.tile_pool(name="psum_y", bufs=2, space="PSUM"))
                yop = stack.enter_context(tc.tile_pool(name="yout", bufs=2))
                # dim1 indexes the six 128-deep contraction sub-tiles; a
                # DoubleRow matmul consumes a [:, 2p:2p+2, :] pair per pass
                xh_sb = iop.tile([128, KT, T], F8, name="xh_sb")
                xl_sb = iop.tile([128, KT, T], F8, name="xl_sb")
                wv_h = iop.tile([128, KT, C], F8, name="wv_h")
                wv_l = iop.tile([128, KT, C], F8, name="wv_l")
                wq_h = iop.tile([128, KT, 2 * C], F8, name="wq_h")
                wq_l = iop.tile([128, KT, 2 * C], F8, name="wq_l")

                # input DMAs: per k-pair, x hi/lo + the v-slice of W hi/lo
                # first (v windows run first and consume digits in this
                # order), then the q/k weights, then the phase-2 masks.
                # The bias tensors slot in where the first evictions need
                # them without delaying the pair stream's front.
                for p in range(NP):
                    sl2 = slice(2 * T * p, 2 * T * (p + 1))
                    slv = slice(2 * C * p, 2 * C * (p + 1))
                    nc.sync.dma_start(out=xh_sb[:, 2 * p:2 * p + 2, :],
                                      in_=xh[:, sl2])
                    nc.sync.dma_start(out=wv_h[:, 2 * p:2 * p + 2, :],
                                      in_=wvh[:, slv])
                    nc.sync.dma_start(out=xl_sb[:, 2 * p:2 * p + 2, :],
                                      in_=xl[:, sl2])
                    nc.sync.dma_start(out=wv_l[:, 2 * p:2 * p + 2, :],
                                      in_=wvl[:, slv])
                    if p == 1:
                        nc.sync.dma_start(out=bvb_sb[:], in_=bvb[:])
                    elif p == 2:
                        nc.sync.dma_start(out=bqk_sb[:], in_=bqk[:])
                # pair 0's q/k weights ship in m0-m3 / m4-m11 halves so
                # the first qk windows start ~1us earlier
                pr0 = slice(0, 2)
                nc.sync.dma_start(out=wq_h[:, pr0, :512], in_=wqh[:, pr0, :512])
                nc.sync.dma_start(out=wq_l[:, pr0, :512], in_=wql[:, pr0, :512])
                nc.sync.dma_start(out=wq_h[:, pr0, 512:], in_=wqh[:, pr0, 512:])
                nc.sync.dma_start(out=wq_l[:, pr0, 512:], in_=wql[:, pr0, 512:])
                for p in range(1, NP):
                    prp = slice(2 * p, 2 * p + 2)
                    nc.sync.dma_start(out=wq_h[:, prp, :], in_=wqh[:, prp, :])
                    nc.sync.dma_start(out=wq_l[:, prp, :], in_=wql[:, prp, :])
                nc.sync.dma_start(out=masks_sb[:], in_=masks[:])
                for s in range(2):
                    nc.gpsimd.memset(attp[s][0][:, 1, 512:640], 0.0)
                    nc.gpsimd.memset(attp[s][1][:, 1, 768:896], 0.0)

                # PE warmup: dummy matmuls on a never-written scratch tile
                # during the initial DMA wait; keeps the HAM activity window
                # busy so the real matmuls start at full clock. Results (and
                # operand garbage) are discarded.
                if WARM_MMS:
                    scratch = iop.tile([128, 512], F16, name="warm_src")
                    nc.vector.memset(scratch[:], 0.0)
                    warm = ps_proj.tile([128, 512], F32, name="ps_warm",
                                        tag="ps_proj")
                    for _ in range(WARM_MMS):
                        nc.tensor.matmul(warm[:], scratch[:, :128], scratch[:],
                                         start=True, stop=True)

                # each group = one [128, 512] PSUM tile (one full bank / zero
                # region) holding one or two 256-wide DoubleRow chunks.
                # ("v", t, n0, width) / ("qk", m, q0, width)
                groups = []
                for t in range(TT):
                    groups.append(("v", t, 0, 512))
                    groups.append(("v", t, 512, 256))
                for m in range(2 * NPAIR):
                    for q0 in (0, 512):
                        groups.append(("qk", m, q0, 512))

                # windows of 4 psum tiles; k-pair-major, digit-product-minor
                # within the window so the PE's consumption order matches the
                # per-pair DMA arrival order (xh, wvh, xl, wvl per pair).
                nwin = (len(groups) + 3) // 4
                for wi, w0 in enumerate(range(0, len(groups), 4)):
                    window = groups[w0:w0 + 4]
                    # borrow the ps_y ring for extra tiles in flight, except
                    # in the last two windows: phase 2's first QK pieces then
                    # find those slots free at the phase boundary
                    borrow = wi < nwin - 2
                    tiles = [(ps_y if (borrow and gi >= 3) else ps_proj).tile(
                                 [128, 512], F32, name="ps_proj",
                                 tag="ps_y" if (borrow and gi >= 3) else "ps_proj")
                             for gi in range(len(window))]
                    nmm = {id(ps): 0 for ps in tiles}
                    total = {id(ps): 9 * (g[3] // 256)
                             for g, ps in zip(window, tiles)}
                    for p in range(NP):
                        pr = slice(2 * p, 2 * p + 2)
                        for term in range(3):
                            xa = (xh_sb, xl_sb, xh_sb)[term]
                            wva = (wv_h, wv_h, wv_l)[term]
                            wqa = (wq_h, wq_h, wq_l)[term]
                            for g, ps in zip(window, tiles):
                                kind, i, o0, wd = g
                                for c0 in range(0, wd, 256):
                                    n = nmm[id(ps)]
                                    nmm[id(ps)] = n + 1
                                    st = n == 0
                                    sp = n == total[id(ps)] - 1
                                    if kind == "v":
                                        nc.tensor.matmul(
                                            ps[:, c0:c0 + 256],
                                            xa[:, pr, 128 * i:128 * (i + 1)],
                                            wva[:, pr, o0 + c0:o0 + c0 + 256],
                                            start=st, stop=sp, perf_mode=DR,
                                        )
                                    else:
                                        nc.tensor.matmul(
                                            ps[:, c0:c0 + 256],
                                            wqa[:, pr, 128 * i:128 * (i + 1)],
                                            xa[:, pr, o0 + c0:o0 + c0 + 256],
                                            start=st, stop=sp, perf_mode=DR,
                                        )
                    for g, ps in zip(window, tiles):
                        kind, i, o0, wd = g
                        if kind == "v":
                            nc.vector.scalar_tensor_tensor(
                                v_sb[i][:, o0:o0 + wd], ps[:, :wd], DESCALE,
                                bvb_sb[:, o0:o0 + wd],
                                AluOpType.mult, AluOpType.add,
                            )
                            if i >= 4:
                                # fp8 digits for the DR AV path, on the
                                # otherwise-idle ACT/Pool engines
                                pb, j2 = (i - 4) // 2, (i - 4) % 2
                                nc.scalar.copy(
                                    v8h[pb][:, j2, o0:o0 + wd],
                                    v_sb[i][:, o0:o0 + wd])
                                nc.gpsimd.tensor_tensor(
                                    v8l[pb][:, j2, o0:o0 + wd],
                                    v_sb[i][:, o0:o0 + wd],
                                    v8h[pb][:, j2, o0:o0 + wd],
                                    AluOpType.subtract)
                        elif i % 2 == 0:
                            nc.scalar.activation(
                                qkT[i][:, o0:o0 + wd], ps[:, :wd],
                                AF.Identity, bias=bqk_sb[:, i:i + 1],
                                scale=DESCALE,
                            )
                        else:
                            nc.vector.tensor_scalar(
                                qkT[i][:, o0:o0 + wd], ps[:, :wd],
                                DESCALE, bqk_sb[:, i:i + 1],
                                AluOpType.mult, AluOpType.add,
                            )

            # ---------- Phase 2: attention, head by head ----------
            # (still inside the io/ps_proj pool scope — no phase barrier)
            if True:
                ps_att = ps_proj
                for a in range(NPAIR):
                    y_pair = yop.tile([128, T], F16, name="y_pair", tag="y_pair")
                    for phase in range(2):   # 0 = QK both heads, 1 = AV both heads
                      for r in range(2):
                        h = 2 * a + r
                        att = attsets[r]
                        qh = qkT[2 * a][64 * r:64 * (r + 1), :]
                        kh = qkT[2 * a + 1][64 * r:64 * (r + 1), :]
                        if phase == 0:
                          # QK^T -> att.T, tk ascending, single-bank psum tiles
                          # for fine-grained slot recycling. Ascending order
                          # ends on the cheap evictions (tk6/7 have no relu),
                          # so AV never waits behind an eviction backlog.
                          for tk in range(TT):
                              k0 = 128 * tk
                              q0d = 256 * (tk // 2)       # start of diag window
                              # piece 1: [k0, 512) if the diag sits left of 512
                              if q0d < 512:
                                  early = h == 0 and tk <= 1
                                  pool2 = ps_y if early else ps_att
                                  ps = pool2.tile([128, 512], F32,
                                                  name="ps_qk",
                                                  tag="ps_y" if early else "ps_proj")
                                  pw = 512 - q0d
                                  nc.tensor.matmul(
                                      ps[:, k0 - q0d:pw], kh[:, k0:k0 + 128],
                                      qh[:, k0:512], start=True, stop=True,
                                  )
                                  # whole piece in one DVE op: relu * [tri|1..]
                                  nc.vector.scalar_tensor_tensor(
                                      att[tk][:, k0:512],
                                      ps[:, k0 - q0d:pw],
                                      0.0, masks_sb[:, :512 - k0],
                                      AluOpType.max, AluOpType.mult,
                                  )
                                  # piece 2: the full [512, 1024) half
                                  ps = pool2.tile([128, 512], F32,
                                                  name="ps_qk",
                                                  tag="ps_y" if early else "ps_proj")
                                  nc.tensor.matmul(
                                      ps[:], kh[:, k0:k0 + 128], qh[:, 512:T],
                                      start=True, stop=True,
                                  )
                                  nc.scalar.activation(att[tk][:, 512:T], ps[:],
                                                       AF.Relu)
                              else:
                                  # single piece [k0, 1024)
                                  ps = ps_att.tile([128, 512], F32,
                                                   name="ps_qk", tag="ps_proj")
                                  pw = T - q0d
                                  nc.tensor.matmul(
                                      ps[:, k0 - q0d:pw], kh[:, k0:k0 + 128],
                                      qh[:, k0:T], start=True, stop=True,
                                  )
                                  nc.vector.scalar_tensor_tensor(
                                      att[tk][:, k0:T],
                                      ps[:, k0 - q0d:pw],
                                      0.0, masks_sb[:, :T - k0],
                                      AluOpType.max, AluOpType.mult,
                                  )

                        if phase == 1:
                          # AV: y.T = v.T @ att.T, groups j ascending, paired
                          # into one [64, 512] psum tile per half; att tiles are
                          # read and released in the order the next head's QK
                          # rewrites them
                          jhis = (3, 1) if h == NH - 1 else (1, 3)
                          yrow = slice(128 * a + 64 * r, 128 * a + 64 * (r + 1))
                          for jhi in jhis:
                              if h == NH - 1 and jhi == 1:
                                  # final half: j=1 then j=0 as separate psum
                                  # groups with their own eviction + DMA on
                                  # alternating engines, so the post-final-
                                  # matmul quantum is one [64,256] copy and
                                  # one 512B-descriptor DMA
                                  for j in (1, 0):
                                      q0 = 256 * j
                                      ntk = 2 * j + 2
                                      ps3 = ps_y.tile([64, 256], F32,
                                                      name="ps_av", tag="ps_y")
                                      for tk in range(ntk):
                                          lo = 128 if tk == 2 * j + 1 else 0
                                          nc.tensor.matmul(
                                              ps3[:, lo:],
                                              v_sb[tk][:, 64 * h:64 * (h + 1)],
                                              att[tk][:, q0 + lo:q0 + 256],
                                              start=(tk == 0),
                                              stop=(tk == ntk - 1),
                                          )
                                      ysl3 = y_pair[64 * r:64 * (r + 1),
                                                    q0:q0 + 256]
                                      nc.vector.tensor_scalar(
                                          ysl3, ps3[:], 0.0, None,
                                          AluOpType.add)
                                  nc.sync.dma_start(
                                      out=yT[yrow, 0:512],
                                      in_=y_pair[64 * r:64 * (r + 1), 0:512])
                                  continue
                              ps2 = ps_y.tile([64, 512], F32, name="ps_av",
                                              tag="ps_y")
                              for j in (jhi - 1, jhi):
                                  q0 = 256 * j
                                  c0 = 256 * (j - (jhi - 1))
                                  ntk = min(TT, 2 * j + 2)
                                  for tk in range(min(ntk, 4)):
                                      lo = 128 if tk == 2 * j + 1 else 0
                                      nc.tensor.matmul(
                                          ps2[:, c0 + lo:c0 + 256],
                                          v_sb[tk][:, 64 * h:64 * (h + 1)],
                                          att[tk][:, q0 + lo:q0 + 256],
                                          start=(tk == 0),
                                          stop=(ntk <= 4 and tk == ntk - 1),
                                      )
                                  npr = (max(ntk, 4) - 4 + 1) // 2
                                  for pb in range(npr):
                                      for dig, v8 in ((0, v8h), (1, v8l)):
                                          nc.tensor.matmul(
                                              ps2[:, c0:c0 + 256],
                                              v8[pb][:, :,
                                                     64 * h:64 * (h + 1)],
                                              attp[r][pb][:, :,
                                                          q0:q0 + 256],
                                              start=False,
                                              stop=(pb == npr - 1
                                                    and dig == 1),
                                              perf_mode=DR,
                                          )
                              ysl = y_pair[64 * r:64 * (r + 1),
                                           256 * (jhi - 1):256 * (jhi + 1)]
                              if a == NPAIR - 1 and r == 0 and jhi == 3:
                                  # spread the last pair's copies over both
                                  # engines: the ACT queue otherwise backs
                                  # up right before the kernel tail
                                  nc.vector.tensor_scalar(
                                      ysl, ps2[:], 0.0, None, AluOpType.add)
                              else:
                                  nc.scalar.copy(ysl, ps2[:])
                              nc.sync.dma_start(
                                  out=yT[yrow,
                                         256 * (jhi - 1):256 * (jhi + 1)],
                                  in_=ysl)

    nc.compile()
    return nc

def _prep_host(x, W_attn, b_attn):
    s = 1.0 / np.sqrt(np.float32(HD))
    W = np.asarray(W_attn, dtype=np.float32).copy()
    b = np.asarray(b_attn, dtype=np.float32).copy()
    W[:C] *= s
    b[:C] *= s
    # interleave q/k head pairs: [q-pair0, k-pair0, q-pair1, k-pair1, ...], v natural
    rows = []
    for a in range(NPAIR):
        rows.extend(range(128 * a, 128 * (a + 1)))          # q heads 2a, 2a+1
        rows.extend(range(C + 128 * a, C + 128 * (a + 1)))  # k heads 2a, 2a+1
    rows.extend(range(2 * C, 3 * C))                        # v natural
    W_perm = W[rows]
    b_perm = b[rows]

    e4 = ml_dtypes.float8_e4m3

    def pack(mat):
        # (C, N) -> partition-major (128, KT*N): each partition's six
        # contraction k-tiles contiguous, k-pair-major
        Cr, N = mat.shape
        return np.ascontiguousarray(
            mat.reshape(KT, 128, N).transpose(1, 0, 2).reshape(128, KT * N))

    def split8(mat):
        hi = mat.astype(e4)
        lo = (mat - hi.astype(np.float32)).astype(e4)
        return hi, lo

    wT = np.ascontiguousarray(W_perm.T) * np.float32(SW)     # (C, 3C)
    wqh, wql = split8(pack(wT[:, :2 * C]).reshape(128, KT, 2 * C))
    wvh, wvl = split8(pack(wT[:, 2 * C:]))
    bqk = np.ascontiguousarray(b_perm[:2 * C].reshape(2 * NPAIR, 128).T)  # (128, 12)
    bvb = np.ascontiguousarray(
        np.broadcast_to(b_perm[2 * C:], (128, C))).astype(np.float16)
    tri = (np.arange(128)[None, :] >= np.arange(128)[:, None]).astype(np.float32)
    masks = np.ones((128, T), dtype=np.float32)
    masks[:, 0:128] = tri          # kept windows always start at the diagonal
    xT = np.asarray(x, dtype=np.float32).transpose(0, 2, 1) * np.float32(SX)  # (B, C, T)
    xhv = np.stack([pack(xT[c]) for c in range(B)])
    xhv, xlv = split8(xhv)
    return xhv, xlv, wqh, wql, wvh, wvl, bqk, bvb, masks


def kernel(x, W_attn, b_attn):
    if "nc" not in _CACHE:
        _CACHE["nc"] = _build()
    nc = _CACHE["nc"]

    xhv, xlv, wqh, wql, wvh, wvl, bqk, bvb, masks = _prep_host(x, W_attn, b_attn)
    in_maps = [
        {"xh": xhv[c], "xl": xlv[c], "wqh": wqh, "wql": wql, "wvh": wvh,
         "wvl": wvl, "bqk": bqk, "bvb": bvb, "masks": masks}
        for c in range(B)
    ]
    res = run_bass_kernel_spmd(nc, in_maps, list(range(B)))
    y = np.empty((B, T, C), dtype=np.float32)
    for c in range(B):
        y[c] = res.results[c]["yT"].T.astype(np.float32)
    return y


# revision 65
# speedup vs baseline: 1.1940x; 1.0050x over previous
"""Trainium2 Bass kernel for causal masked-ReLU attention (no softmax).

Reference computation (B=8, T=1024, C=768, n_head=12, hd=64):
    qkv = x @ W_attn.T + b_attn
    q, k, v = split(qkv); per-head: att = relu(mask_causal(q k^T / sqrt(hd)))
    y = att @ v, heads re-merged -> (B, T, C)

Sharding: one batch element per NeuronCore (8 cores). Each core computes the
QKV projection and all 12 heads' attention for its batch element.

Layout strategy (per core):
  - Host passes x[b].T (C, T) and W.T (C, 3C) so the contraction dim C lands
    on SBUF partitions with unit-stride DMA (no on-chip transposes).
  - W rows are pre-permuted on host into [q-pair0, k-pair0, q-pair1, ...] so
    q.T / k.T of head h live at the same partition offset (h%2)*64 of their
    M-tiles; matmul operands then share a base partition.
  - q weights/bias are pre-scaled by 1/sqrt(hd) on host.
  - QKV projection runs in fp8 (e4m3) DoubleRow perf mode: 256-deep
    contraction per pass at 0.5 cycles/row = 4x fp16 PE throughput. Operands
    are split into hi+lo fp8 digits (x = xh + xl, W = wh + wl, both
    pre-scaled into e4m3's normal range) and three digit products
    xh*wh + xl*wh + xh*wl accumulate in the same fp32 PSUM group; the
    dropped xl*wl term is ~1e-4 relative. The 2^13 operand scaling is
    removed at eviction (activation scale / tensor_scalar multiply), where
    the bias is also added. Net error ~1.2e-3, PE cost 0.75x of fp16.
  - att is computed transposed (att.T = k @ q.T, layout [T_k, T_q]) so the AV
    matmul (y.T = v.T @ att.T) streams att.T directly with v as stationary.
    QK runs in fp16. AV splits by causal depth: att tiles 0-3 (72% of y's
    variance) stay fp16; tiles 4-7 (~11% variance share - they only serve
    long causal rows) are stored 1-digit fp8 and consumed in DoubleRow
    pair matmuls against 2-digit fp8 v (digits produced on the idle
    ACT/Pool engines in phase 1), contributing ~0.9% error for a 2.5us PE
    saving. Full-fp8 att (2.7% error) or 2-digit att (eviction traffic
    doubles) both fail their budgets.
  - Causal structure at 128-col granularity: fully-masked regions are never
    computed or read except two 128-wide strips the fp8 DR pairs span,
    zero-filled once at startup.
  - All fp8 operands are packed partition-major on the host so each k-pair
    (or whole digit tensor) moves in ONE DMA with 2KB+ descriptor runs: the
    HWDGE's fixed ~625ns per-DMA cost otherwise serializes the input stream
    (19 input DMAs total, ordered v-projection digits first).
  - One pool scope spans both phases (a pool close = all-engine barrier);
    projection windows borrow the spare psum ring for 8 tiles in flight,
    except the last two windows, whose spare slots phase 2's first QK
    pieces pick up barrier-free.
  - QK evictions and AV accumulation groups both run ascending, so att
    tiles are read and released in the same order the next head rewrites
    them, and the QK eviction stream ends on the cheap diagonal tiles.
  - Eviction work (masked-ReLU, bias adds, y copies) is balanced across
    the ACT and DVE engines, which sit just under the PE's per-head time;
    the final head's last AV group is split into [64,256] quarters with
    copies on both engines so the closing copy+DMA+sem chain is minimal.
  - Output is written as y.T (C, T) in fp16; host transposes and upcasts.
"""

import numpy as np

import sys
for _p in ("/opt/trn_rl_repo", "/root/.axon_site", "/root/.axon_site/_ro/trn_rl_repo",
           "/root/.axon_site/_ro/pypackages"):
    if _p not in sys.path:
        sys.path.append(_p)

import ml_dtypes

import concourse.bacc as bacc
import concourse.mybir as mybir
from concourse.alu_op_type import AluOpType
from concourse.tile import TileContext
from concourse.tile_rust import add_dep_helper
from concourse.bass_utils import run_bass_kernel_spmd

B, T, C = 8, 1024, 768
NH, HD = 12, 64
C3 = 3 * C            # 2304
KT = C // 128         # 6  contraction tiles of the projection
NP = KT // 2          # 3  contraction pairs (DoubleRow)
TT = T // 128         # 8  tiles of the sequence dim
NPAIR = NH // 2       # 6  head pairs
NW = T // 256         # 4  256-wide attention windows
F32 = mybir.dt.float32
F16 = mybir.dt.float16
F8 = mybir.dt.float8e4
AF = mybir.ActivationFunctionType
DR = mybir.MatmulPerfMode.DoubleRow

SX = 16.0             # x pre-scale (keeps x-lo digits in e4m3 normal range)
SW = 512.0            # W pre-scale
DESCALE = 1.0 / (SX * SW)

WARM_MMS = 0

_CACHE = {}


def _build():
    nc = bacc.Bacc("TRN2", target_bir_lowering=False, debug=False, num_devices=8)

    # host packs all fp8 operands partition-major ([128, ...] with each
    # partition's six k-tiles contiguous) so one DMA moves a whole k-pair
    # (or tensor) with 2KB+ descriptor runs: the HWDGE's fixed ~625ns cost
    # per DMA dominates the input stream otherwise
    xh = nc.dram_tensor("xh", [128, KT * T], F8, kind="ExternalInput").ap()
    xl = nc.dram_tensor("xl", [128, KT * T], F8, kind="ExternalInput").ap()
    wvh = nc.dram_tensor("wvh", [128, KT * C], F8, kind="ExternalInput").ap()
    wvl = nc.dram_tensor("wvl", [128, KT * C], F8, kind="ExternalInput").ap()
    wqh = nc.dram_tensor("wqh", [128, KT, 2 * C], F8, kind="ExternalInput").ap()
    wql = nc.dram_tensor("wql", [128, KT, 2 * C], F8, kind="ExternalInput").ap()
    bqk = nc.dram_tensor("bqk", [128, 2 * NPAIR], F32, kind="ExternalInput").ap()
    bvb = nc.dram_tensor("bvb", [128, C], F16, kind="ExternalInput").ap()
    # masks = [tri(128) | ones(896)]: the kept region of att.T tile tk always
    # starts with the triangular diagonal block, so masks[:, :width] is the
    # relu-mask for any kept window
    masks = nc.dram_tensor("masks", [128, T], F32, kind="ExternalInput").ap()
    masks2 = nc.dram_tensor("masks2", [128, 256], F32, kind="ExternalInput").ap()
    yT = nc.dram_tensor("yT", [C, T], F16, kind="ExternalOutput").ap()

    with TileContext(nc) as tc:
        with (
            tc.tile_pool(name="persist", bufs=1) as pp,
        ):
            masks_sb = pp.tile([128, T], F32, name="masks_sb")
            masks2_sb = pp.tile([128, 256], F32, name="masks2_sb")
            # duplicated 1-digit fp8 copies of q/k columns 768:1024 (dim1 =
            # two identical slots, the DoubleRow subtile pair): QK tiles 6/7
            # then run as one DR matmul producing 2x att, rescaled by masks2
            q8d = [pp.tile([128, 2, 256], F8, name=f"q8d{i}")
                   for i in range(NPAIR)]
            k8d = [pp.tile([128, 2, 256], F8, name=f"k8d{i}")
                   for i in range(NPAIR)]
            bqk_sb = pp.tile([128, 2 * NPAIR], F32, name="bqk_sb")
            bvb_sb = pp.tile([128, C], F16, name="bvb_sb")
            qkT = [pp.tile([128, T], F16, name=f"qkT{m}") for m in range(2 * NPAIR)]
            v_sb = [pp.tile([128, C], F16, name=f"v{t}") for t in range(TT)]
            # att tiles 0-3 fp16; tiles 4-7 live as fp8 DoubleRow pair-tiles
            # (dim1 = tile parity). Their ~11% variance share of y keeps the
            # 1-digit fp8 error contribution ~0.9%.
            att16 = [[pp.tile([128, T], F16, name=f"att{s}_{t}")
                      for t in range(4)] for s in range(2)]
            attp = [[pp.tile([128, 2, T], F8, name=f"attp{s}_{pb}")
                     for pb in range(2)] for s in range(2)]
            # v8: on-chip hi/lo fp8 digits of v tiles 4-7, pair-packed
            v8h = [pp.tile([128, 2, C], F8, name=f"v8h{pb}") for pb in range(2)]
            v8l = [pp.tile([128, 2, C], F8, name=f"v8l{pb}") for pb in range(2)]
            attsets = [att16[s] + [attp[s][pb][:, j2, :]
                                   for pb in range(2) for j2 in range(2)]
                       for s in range(2)]

            # ---------- Phase 1: QKV projection (fp8 DoubleRow, 3 digit
            # products xh*wh + xl*wh + xh*wl into one PSUM group) ----------
            # The io/psum pools deliberately stay open across both phases:
            # closing a pool inserts an all-engine barrier that idles the PE
            # for ~1.5us at the phase boundary. Phase 2's QK psum tiles come
            # from the same rotation, so the first heads naturally pipeline
            # behind the last projection windows.
            from contextlib import ExitStack
            with ExitStack() as stack:
                iop = stack.enter_context(tc.tile_pool(name="io", bufs=1))
                # one 8-slot ring covering all PSUM use in both phases: more
                # projection tiles in flight (the stall there is tiles-in-
                # flight-bound while the input DMAs stream), and no pool
                # barrier between phases
                ps_proj = stack.enter_context(
                    tc.tile_pool(name="psum_proj", bufs=6, space="PSUM"))
                ps_y = stack.enter_context(
                    tc.tile_pool(name="psum_y", bufs=2, space="PSUM"))
                yop = stack.enter_context(tc.tile_pool(name="yout", bufs=2))
                # dim1 indexes the six 128-deep contraction sub-tiles; a
                # DoubleRow matmul consumes a [:, 2p:2p+2, :] pair per pass
                xh_sb = iop.tile([128, KT, T], F8, name="xh_sb")
                xl_sb = iop.tile([128, KT, T], F8, name="xl_sb")
                wv_h = iop.tile([128, KT, C], F8, name="wv_h")
                wv_l = iop.tile([128, KT, C], F8, name="wv_l")
                wq_h = iop.tile([128, KT, 2 * C], F8, name="wq_h")
                wq_l = iop.tile([128, KT, 2 * C], F8, name="wq_l")

                # input DMAs: per k-pair, x hi/lo + the v-slice of W hi/lo
                # first (v windows run first and consume digits in this
                # order), then the q/k weights, then the phase-2 masks.
                # The bias tensors slot in where the first evictions need
                # them without delaying the pair stream's front.
                for p in range(NP):
                    sl2 = slice(2 * T * p, 2 * T * (p + 1))
                    slv = slice(2 * C * p, 2 * C * (p + 1))
                    nc.sync.dma_start(out=xh_sb[:, 2 * p:2 * p + 2, :],
                                      in_=xh[:, sl2])
                    nc.sync.dma_start(out=wv_h[:, 2 * p:2 * p + 2, :],
                                      in_=wvh[:, slv])
                    nc.sync.dma_start(out=xl_sb[:, 2 * p:2 * p + 2, :],
                                      in_=xl[:, sl2])
                    nc.sync.dma_start(out=wv_l[:, 2 * p:2 * p + 2, :],
                                      in_=wvl[:, slv])
                    if p == 1:
                        nc.sync.dma_start(out=bvb_sb[:], in_=bvb[:])
                    elif p == 2:
                        nc.sync.dma_start(out=bqk_sb[:], in_=bqk[:])
                # pair 0's q/k weights ship in m0-m3 / m4-m11 halves so
                # the first qk windows start ~1us earlier
                pr0 = slice(0, 2)
                nc.sync.dma_start(out=wq_h[:, pr0, :512], in_=wqh[:, pr0, :512])
                nc.sync.dma_start(out=wq_l[:, pr0, :512], in_=wql[:, pr0, :512])
                nc.sync.dma_start(out=wq_h[:, pr0, 512:], in_=wqh[:, pr0, 512:])
                nc.sync.dma_start(out=wq_l[:, pr0, 512:], in_=wql[:, pr0, 512:])
                for p in range(1, NP):
                    prp = slice(2 * p, 2 * p + 2)
                    nc.sync.dma_start(out=wq_h[:, prp, :], in_=wqh[:, prp, :])
                    nc.sync.dma_start(out=wq_l[:, prp, :], in_=wql[:, prp, :])
                nc.sync.dma_start(out=masks_sb[:], in_=masks[:])
                nc.sync.dma_start(out=masks2_sb[:], in_=masks2[:])
                for s in range(2):
                    nc.gpsimd.memset(attp[s][0][:, 1, 512:640], 0.0)
                    nc.gpsimd.memset(attp[s][1][:, 1, 768:896], 0.0)

                # PE warmup: dummy matmuls on a never-written scratch tile
                # during the initial DMA wait; keeps the HAM activity window
                # busy so the real matmuls start at full clock. Results (and
                # operand garbage) are discarded.
                if WARM_MMS:
                    scratch = iop.tile([128, 512], F16, name="warm_src")
                    nc.vector.memset(scratch[:], 0.0)
                    warm = ps_proj.tile([128, 512], F32, name="ps_warm",
                                        tag="ps_proj")
                    for _ in range(WARM_MMS):
                        nc.tensor.matmul(warm[:], scratch[:, :128], scratch[:],
                                         start=True, stop=True)

                # each group = one [128, 512] PSUM tile (one full bank / zero
                # region) holding one or two 256-wide DoubleRow chunks.
                # ("v", t, n0, width) / ("qk", m, q0, width)
                groups = []
                for t in range(TT):
                    groups.append(("v", t, 0, 512))
                    groups.append(("v", t, 512, 256))
                for m in range(2 * NPAIR):
                    for q0 in (0, 512):
                        groups.append(("qk", m, q0, 512))

                # windows of 4 psum tiles; k-pair-major, digit-product-minor
                # within the window so the PE's consumption order matches the
                # per-pair DMA arrival order (xh, wvh, xl, wvl per pair).
                nwin = (len(groups) + 3) // 4
                for wi, w0 in enumerate(range(0, len(groups), 4)):
                    window = groups[w0:w0 + 4]
                    # borrow the ps_y ring for extra tiles in flight, except
                    # in the last two windows: phase 2's first QK pieces then
                    # find those slots free at the phase boundary
                    borrow = wi < nwin - 2
                    tiles = [(ps_y if (borrow and gi >= 3) else ps_proj).tile(
                                 [128, 512], F32, name="ps_proj",
                                 tag="ps_y" if (borrow and gi >= 3) else "ps_proj")
                             for gi in range(len(window))]
                    nmm = {id(ps): 0 for ps in tiles}
                    total = {id(ps): 9 * (g[3] // 256)
                             for g, ps in zip(window, tiles)}
                    for p in range(NP):
                        pr = slice(2 * p, 2 * p + 2)
                        for term in range(3):
                            xa = (xh_sb, xl_sb, xh_sb)[term]
                            wva = (wv_h, wv_h, wv_l)[term]
                            wqa = (wq_h, wq_h, wq_l)[term]
                            for g, ps in zip(window, tiles):
                                kind, i, o0, wd = g
                                for c0 in range(0, wd, 256):
                                    n = nmm[id(ps)]
                                    nmm[id(ps)] = n + 1
                                    st = n == 0
                                    sp = n == total[id(ps)] - 1
                                    if kind == "v":
                                        nc.tensor.matmul(
                                            ps[:, c0:c0 + 256],
                                            xa[:, pr, 128 * i:128 * (i + 1)],
                                            wva[:, pr, o0 + c0:o0 + c0 + 256],
                                            start=st, stop=sp, perf_mode=DR,
                                        )
                                    else:
                                        nc.tensor.matmul(
                                            ps[:, c0:c0 + 256],
                                            wqa[:, pr, 128 * i:128 * (i + 1)],
                                            xa[:, pr, o0 + c0:o0 + c0 + 256],
                                            start=st, stop=sp, perf_mode=DR,
                                        )
                    for g, ps in zip(window, tiles):
                        kind, i, o0, wd = g
                        if kind == "v":
                            nc.vector.scalar_tensor_tensor(
                                v_sb[i][:, o0:o0 + wd], ps[:, :wd], DESCALE,
                                bvb_sb[:, o0:o0 + wd],
                                AluOpType.mult, AluOpType.add,
                            )
                            if i >= 4:
                                # fp8 digits for the DR AV path, on the
                                # otherwise-idle ACT/Pool engines
                                pb, j2 = (i - 4) // 2, (i - 4) % 2
                                nc.scalar.copy(
                                    v8h[pb][:, j2, o0:o0 + wd],
                                    v_sb[i][:, o0:o0 + wd])
                                nc.gpsimd.tensor_tensor(
                                    v8l[pb][:, j2, o0:o0 + wd],
                                    v_sb[i][:, o0:o0 + wd],
                                    v8h[pb][:, j2, o0:o0 + wd],
                                    AluOpType.subtract)
                        elif i % 2 == 0:
                            nc.scalar.activation(
                                qkT[i][:, o0:o0 + wd], ps[:, :wd],
                                AF.Identity, bias=bqk_sb[:, i:i + 1],
                                scale=DESCALE,
                            )
                            if o0 == 512:
                                nc.scalar.copy(q8d[i // 2][:, 0, :],
                                               qkT[i][:, 768:])
                                nc.gpsimd.tensor_scalar(
                                    q8d[i // 2][:, 1, :], qkT[i][:, 768:],
                                    0.0, None, AluOpType.add)
                        else:
                            nc.vector.tensor_scalar(
                                qkT[i][:, o0:o0 + wd], ps[:, :wd],
                                DESCALE, bqk_sb[:, i:i + 1],
                                AluOpType.mult, AluOpType.add,
                            )
                            if o0 == 512:
                                nc.scalar.copy(k8d[i // 2][:, 0, :],
                                               qkT[i][:, 768:])
                                nc.gpsimd.tensor_scalar(
                                    k8d[i // 2][:, 1, :], qkT[i][:, 768:],
                                    0.0, None, AluOpType.add)

            # ---------- Phase 2: attention, head by head ----------
            # (still inside the io/ps_proj pool scope — no phase barrier)
            if True:
                ps_att = ps_proj
                for a in range(NPAIR):
                    y_pair = yop.tile([128, T], F16, name="y_pair", tag="y_pair")
                    for phase in range(2):   # 0 = QK both heads, 1 = AV both heads
                      for r in range(2):
                        h = 2 * a + r
                        att = attsets[r]
                        qh = qkT[2 * a][64 * r:64 * (r + 1), :]
                        kh = qkT[2 * a + 1][64 * r:64 * (r + 1), :]
                        if phase == 0:
                          # QK^T -> att.T, tk ascending, single-bank psum tiles
                          # for fine-grained slot recycling. Ascending order
                          # ends on the cheap evictions (tk6/7 have no relu),
                          # so AV never waits behind an eviction backlog.
                          for tk in range(TT):
                              k0 = 128 * tk
                              q0d = 256 * (tk // 2)       # start of diag window
                              # piece 1: [k0, 512) if the diag sits left of 512
                              if q0d < 512:
                                  early = h == 0 and tk <= 1
                                  pool2 = ps_y if early else ps_att
                                  ps = pool2.tile([128, 512], F32,
                                                  name="ps_qk",
                                                  tag="ps_y" if early else "ps_proj")
                                  pw = 512 - q0d
                                  nc.tensor.matmul(
                                      ps[:, k0 - q0d:pw], kh[:, k0:k0 + 128],
                                      qh[:, k0:512], start=True, stop=True,
                                  )
                                  # whole piece in one DVE op: relu * [tri|1..]
                                  nc.vector.scalar_tensor_tensor(
                                      att[tk][:, k0:512],
                                      ps[:, k0 - q0d:pw],
                                      0.0, masks_sb[:, :512 - k0],
                                      AluOpType.max, AluOpType.mult,
                                  )
                                  # piece 2: the full [512, 1024) half
                                  ps = pool2.tile([128, 512], F32,
                                                  name="ps_qk",
                                                  tag="ps_y" if early else "ps_proj")
                                  nc.tensor.matmul(
                                      ps[:], kh[:, k0:k0 + 128], qh[:, 512:T],
                                      start=True, stop=True,
                                  )
                                  nc.scalar.activation(att[tk][:, 512:T], ps[:],
                                                       AF.Relu)
                              else:
                                  # single piece [k0, 1024); tiles 6/7 run as
                                  # one fp8 DoubleRow matmul on duplicated
                                  # 1-digit q/k (result is 2x; the 0.5-scaled
                                  # masks2 folds it back at eviction)
                                  ps = ps_att.tile([128, 512], F32,
                                                   name="ps_qk", tag="ps_proj")
                                  pw = T - q0d
                                  if tk >= 6:
                                      c8 = k0 - 768
                                      nc.tensor.matmul(
                                          ps[:, k0 - q0d:pw],
                                          k8d[a][64 * r:64 * (r + 1), :,
                                                 c8:c8 + 128],
                                          q8d[a][64 * r:64 * (r + 1), :,
                                                 c8:],
                                          start=True, stop=True,
                                          perf_mode=DR,
                                      )
                                      nc.vector.scalar_tensor_tensor(
                                          att[tk][:, k0:T],
                                          ps[:, k0 - q0d:pw],
                                          0.0, masks2_sb[:, :T - k0],
                                          AluOpType.max, AluOpType.mult,
                                      )
                                  else:
                                      nc.tensor.matmul(
                                          ps[:, k0 - q0d:pw],
                                          kh[:, k0:k0 + 128],
                                          qh[:, k0:T], start=True, stop=True,
                                      )
                                      nc.vector.scalar_tensor_tensor(
                                          att[tk][:, k0:T],
                                          ps[:, k0 - q0d:pw],
                                          0.0, masks_sb[:, :T - k0],
                                          AluOpType.max, AluOpType.mult,
                                      )

                        if phase == 1:
                          # AV: y.T = v.T @ att.T, groups j ascending, paired
                          # into one [64, 512] psum tile per half; att tiles are
                          # read and released in the order the next head's QK
                          # rewrites them
                          jhis = (3, 1) if h == NH - 1 else (1, 3)
                          yrow = slice(128 * a + 64 * r, 128 * a + 64 * (r + 1))
                          for jhi in jhis:
                              if h == NH - 1 and jhi == 1:
                                  # final half: j=1 then j=0 as separate psum
                                  # groups with their own eviction + DMA on
                                  # alternating engines, so the post-final-
                                  # matmul quantum is one [64,256] copy and
                                  # one 512B-descriptor DMA
                                  for j in (1, 0):
                                      q0 = 256 * j
                                      ntk = 2 * j + 2
                                      ps3 = ps_y.tile([64, 256], F32,
                                                      name="ps_av", tag="ps_y")
                                      for tk in range(ntk):
                                          lo = 128 if tk == 2 * j + 1 else 0
                                          nc.tensor.matmul(
                                              ps3[:, lo:],
                                              v_sb[tk][:, 64 * h:64 * (h + 1)],
                                              att[tk][:, q0 + lo:q0 + 256],
                                              start=(tk == 0),
                                              stop=(tk == ntk - 1),
                                          )
                                      ysl3 = y_pair[64 * r:64 * (r + 1),
                                                    q0:q0 + 256]
                                      nc.vector.tensor_scalar(
                                          ysl3, ps3[:], 0.0, None,
                                          AluOpType.add)
                                  nc.sync.dma_start(
                                      out=yT[yrow, 0:512],
                                      in_=y_pair[64 * r:64 * (r + 1), 0:512])
                                  continue
                              ps2 = ps_y.tile([64, 512], F32, name="ps_av",
                                              tag="ps_y")
                              for j in (jhi - 1, jhi):
                                  q0 = 256 * j
                                  c0 = 256 * (j - (jhi - 1))
                                  ntk = min(TT, 2 * j + 2)
                                  for tk in range(min(ntk, 4)):
                                      lo = 128 if tk == 2 * j + 1 else 0
                                      nc.tensor.matmul(
                                          ps2[:, c0 + lo:c0 + 256],
                                          v_sb[tk][:, 64 * h:64 * (h + 1)],
                                          att[tk][:, q0 + lo:q0 + 256],
                                          start=(tk == 0),
                                          stop=(ntk <= 4 and tk == ntk - 1),
                                      )
                                  npr = (max(ntk, 4) - 4 + 1) // 2
                                  for pb in range(npr):
                                      for dig, v8 in ((0, v8h), (1, v8l)):
                                          nc.tensor.matmul(
                                              ps2[:, c0:c0 + 256],
                                              v8[pb][:, :,
                                                     64 * h:64 * (h + 1)],
                                              attp[r][pb][:, :,
                                                          q0:q0 + 256],
                                              start=False,
                                              stop=(pb == npr - 1
                                                    and dig == 1),
                                              perf_mode=DR,
                                          )
                              ysl = y_pair[64 * r:64 * (r + 1),
                                           256 * (jhi - 1):256 * (jhi + 1)]
                              if a == NPAIR - 1 and r == 0 and jhi == 3:
                                  # spread the last pair's copies over both
                                  # engines: the ACT queue otherwise backs
                                  # up right before the kernel tail
                                  nc.vector.tensor_scalar(
                                      ysl, ps2[:], 0.0, None, AluOpType.add)
                              else:
                                  nc.scalar.copy(ysl, ps2[:])
                              nc.sync.dma_start(
                                  out=yT[yrow,
                                         256 * (jhi - 1):256 * (jhi + 1)],
                                  in_=ysl)

    nc.compile()
    return nc

def _prep_host(x, W_attn, b_attn):
    s = 1.0 / np.sqrt(np.float32(HD))
    W = np.asarray(W_attn, dtype=np.float32).copy()
    b = np.asarray(b_attn, dtype=np.float32).copy()
    W[:C] *= s
    b[:C] *= s
    # interleave q/k head pairs: [q-pair0, k-pair0, q-pair1, k-pair1, ...], v natural
    rows = []
    for a in range(NPAIR):
        rows.extend(range(128 * a, 128 * (a + 1)))          # q heads 2a, 2a+1
        rows.extend(range(C + 128 * a, C + 128 * (a + 1)))  # k heads 2a, 2a+1
    rows.extend(range(2 * C, 3 * C))                        # v natural
    W_perm = W[rows]
    b_perm = b[rows]

    e4 = ml_dtypes.float8_e4m3

    def pack(mat):
        # (C, N) -> partition-major (128, KT*N): each partition's six
        # contraction k-tiles contiguous, k-pair-major
        Cr, N = mat.shape
        return np.ascontiguousarray(
            mat.reshape(KT, 128, N).transpose(1, 0, 2).reshape(128, KT * N))

    def split8(mat):
        hi = mat.astype(e4)
        lo = (mat - hi.astype(np.float32)).astype(e4)
        return hi, lo

    wT = np.ascontiguousarray(W_perm.T) * np.float32(SW)     # (C, 3C)
    wqh, wql = split8(pack(wT[:, :2 * C]).reshape(128, KT, 2 * C))
    wvh, wvl = split8(pack(wT[:, 2 * C:]))
    bqk = np.ascontiguousarray(b_perm[:2 * C].reshape(2 * NPAIR, 128).T)  # (128, 12)
    bvb = np.ascontiguousarray(
        np.broadcast_to(b_perm[2 * C:], (128, C))).astype(np.float16)
    tri = (np.arange(128)[None, :] >= np.arange(128)[:, None]).astype(np.float32)
    masks = np.ones((128, T), dtype=np.float32)
    masks[:, 0:128] = tri          # kept windows always start at the diagonal
    masks2 = np.ascontiguousarray(0.5 * masks[:, :256])
    xT = np.asarray(x, dtype=np.float32).transpose(0, 2, 1) * np.float32(SX)  # (B, C, T)
    xhv = np.stack([pack(xT[c]) for c in range(B)])
    xhv, xlv = split8(xhv)
    return xhv, xlv, wqh, wql, wvh, wvl, bqk, bvb, masks, masks2


def kernel(x, W_attn, b_attn):
    if "nc" not in _CACHE:
        _CACHE["nc"] = _build()
    nc = _CACHE["nc"]

    (xhv, xlv, wqh, wql, wvh, wvl, bqk, bvb, masks,
     masks2) = _prep_host(x, W_attn, b_attn)
    in_maps = [
        {"xh": xhv[c], "xl": xlv[c], "wqh": wqh, "wql": wql, "wvh": wvh,
         "wvl": wvl, "bqk": bqk, "bvb": bvb, "masks": masks,
         "masks2": masks2}
        for c in range(B)
    ]
    res = run_bass_kernel_spmd(nc, in_maps, list(range(B)))
    y = np.empty((B, T, C), dtype=np.float32)
    for c in range(B):
        y[c] = res.results[c]["yT"].T.astype(np.float32)
    return y
